# revision 24
# baseline (speedup 1.0000x reference)
"""LPCNet sampling kernel for Trainium2 — nn_LPCNet_91061896609827.

kernel(**inputs) takes FULL unsharded inputs (as from reference.setup_inputs())
and returns the FULL [B, F*T, K] float32 logits output.

Strategy: data-parallel over the R = B*F = 1024 row axis, 8 shards of 128 rows
(one per NeuronCore, rows on SBUF partitions), GRU/dense weights replicated.
The T=160 sequential sampling scan runs fully on-device per core.

v2 vs v1 (5.01ms):
  - gumbel g = ln(-ln u) precomputed on HOST; no device prepass, no u DMA.
  - all recurrent matmuls single-pass bf16 (fp32 matmul = 2 PE passes);
    dyn (p/s/e feature) matmul keeps exact bf16 hi/lo split.
  - static gate biases (cond@Wxa+ba, bb) folded into PSUM accumulation via
    identity/ones matmuls -> sigmoid/tanh read PSUM directly, the wide
    vector adds disappear from the serial chain.
  - garz accumulation reordered: the big wha matmuls for step t+1 are
    emitted at the end of step t (hidden under GRU-B/sampling); only the
    tiny dyn matmuls sit on the critical path.
  - round(soft) via direct f32->i32 cast (hardware round-half-even ==
    jnp.round; verified by probe).
  - sigmoid LUT (one op) instead of tanh(0.5x) rescaling (three ops);
    r/z sigmoids split so r is ready earlier.
  - ha transpose copies spread across scalar/vector/gpsimd engines.

Self-contained: hardcodes shapes; reads nothing from /root/problem.
"""
import numpy as np
from contextlib import ExitStack

import concourse.bass as bass
import concourse.tile as tile
import concourse.mybir as mybir
from concourse import bacc
from concourse.bass_utils import run_bass_kernel_spmd

B, F, M, NF = 32, 32, 16, 20
T, K = 160, 256
R = B * F
COND, HA, HB = 128, 384, 16
N_CORES = 8
P = 128  # rows per core == SBUF partitions

f32 = mybir.dt.float32
bf16 = mybir.dt.bfloat16
i32 = mybir.dt.int32
f32r = mybir.dt.float32r
AF = mybir.ActivationFunctionType
ALU = mybir.AluOpType

G3 = 3 * HA  # 1152


def _build(T_steps: int, dbg: bool = False):
    nc = bacc.Bacc("TRN2", target_bir_lowering=False, debug=False,
                   num_devices=N_CORES)

    def din(name, shape, dt=f32):
        return nc.dram_tensor(name, list(shape), dt, kind="ExternalInput").ap()

    dbg_outs = {}
    if dbg:
        for nm, sh in [("d_soft0", [P, 1]), ("d_e0", [P, 1]), ("d_s0", [P, 1]),
                       ("d_p0", [P, 1]), ("d_p1", [P, 1]), ("d_pdot1", [P, 1]),
                       ("d_ha0", [P, HA]), ("d_hb0", [P, HB]),
                       ("d_garz1", [P, 768]), ("d_ghn1", [P, HA]),
                       ("d_gxn1", [P, HA]), ("d_r1", [P, HA]),
                       ("d_an1", [P, HA]), ("d_ha1", [P, HA]),
                       ("d_gb1", [P, 64]), ("d_hb1", [P, HB]),
                       ("d_dynT1", [3, P]), ("d_hbTa1", [HB + 1, P]),
                       ("d_gb1o", [P, 64])]:
            dbg_outs[nm] = nc.dram_tensor(nm, sh, f32, kind="ExternalOutput").ap()

    ins = {
        # per-core
        "feat": din("feat", [P, NF]),
        "lpcrot": din("lpcrot", [P, 16 * 16]),
        "gl": din("gl", [T_steps, P, K]),          # ln(-ln u), host-computed
        # replicated statics (host-packed)
        "wf1b": din("wf1b", [NF + 1, COND]),
        "wf2": din("wf2", [COND, COND]),
        "bf2": din("bf2", [1, COND]),
        "wxac": din("wxac", [COND, G3]),
        "ba": din("ba", [1, G3]),
        "wdynh": din("wdynh", [3, G3], bf16),
        "wdynl": din("wdynl", [3, G3], bf16),
        "wha": din("wha", [P, 3 * G3], f32r),      # 3 K-chunks on free axis
        "wxb": din("wxb", [P, 3 * 48], f32r),      # chunk c: [rz 32 | n 16]
        "whbbx": din("whbbx", [HB + 1, 64], f32r),  # [[Whb_rz;bb_rz] | [0;bb_n] | [Whb_n;0]]
        "w12b": din("w12b", [HB + 1, 2 * K], f32r),  # [W1|W2 ; b1|b2]
        "g12": din("g12", [P, 2 * K]),             # [g1|g2] row broadcast
        "idxb": din("idxb", [P, K]),               # iota row broadcast
        "ident": din("ident", [P, P]),
        "identr": din("identr", [P, P], f32r),
        "onesrow": din("onesrow", [1, P]),
        "onesrowr": din("onesrowr", [1, P], f32r),
        "zhaT": din("zhaT", [P, 3 * P], f32r),
        "zhbT": din("zhbT", [HB, P], f32r),
        "taub": din("taub", [P, K]),
    }
    out_l = nc.dram_tensor("logits", [T_steps, P, K], f32,
                           kind="ExternalOutput").ap()

    with tile.TileContext(nc) as tc, ExitStack() as ctx:
        st = ctx.enter_context(tc.tile_pool(name="static", bufs=1))
        wk = ctx.enter_context(tc.tile_pool(name="work", bufs=2))
        io = ctx.enter_context(tc.tile_pool(name="io", bufs=3))
        ps_rz = ctx.enter_context(tc.tile_pool(name="ps_rz", bufs=1, space="PSUM"))
        ps_n = ctx.enter_context(tc.tile_pool(name="ps_n", bufs=1, space="PSUM"))
        ps_x = ctx.enter_context(tc.tile_pool(name="ps_x", bufs=1, space="PSUM"))
        ps_b = ctx.enter_context(tc.tile_pool(name="ps_b", bufs=1, space="PSUM"))
        ps_l = ctx.enter_context(tc.tile_pool(name="ps_l", bufs=1, space="PSUM"))
        ps_t = ctx.enter_context(tc.tile_pool(name="ps_t", bufs=1, space="PSUM"))
        ps_w = ctx.enter_context(tc.tile_pool(name="ps_w", bufs=1, space="PSUM"))

        # ---- load statics ----
        def load(name, shape, dt=f32, tag=None):
            t_ = st.tile(list(shape), dt, tag=tag or name)
            nc.sync.dma_start(t_[:], ins[name][:])
            return t_

        feat = load("feat", [P, NF])
        lpcrot = load("lpcrot", [P, 256])
        wf1b = load("wf1b", [NF + 1, COND])
        wf2 = load("wf2", [COND, COND])
        bf2 = load("bf2", [1, COND])
        wxac = load("wxac", [COND, G3])
        ba = load("ba", [1, G3])
        wdynh = load("wdynh", [3, G3], bf16)
        wdynl = load("wdynl", [3, G3], bf16)
        wha = load("wha", [P, 3 * G3], f32r)
        wxb = load("wxb", [P, 3 * 48], f32r)
        whbbx = load("whbbx", [HB + 1, 64], f32r)
        w12b = load("w12b", [HB + 1, 2 * K], f32r)
        g12 = load("g12", [P, 2 * K])
        idxb = load("idxb", [P, K])
        ident = load("ident", [P, P])
        identr = load("identr", [P, P], f32r)
        taub = load("taub", [P, K])

        # ---- persistent state ----
        haTw = st.tile([P, 3 * P], f32r, tag="haTw")
        haT = [haTw[:, c * P:(c + 1) * P] for c in range(3)]
        hbTa = st.tile([HB + 1, P], f32r, tag="hbTa")
        ring = st.tile([P, 16], f32, tag="ring")
        dynstage = st.tile([P, 3], f32, tag="dynstage")
        gxs = st.tile([P, G3], f32, tag="gxs")
        gxsr = st.tile([P, G3], f32r, tag="gxsr")

        nc.sync.dma_start(haTw[:], ins["zhaT"][:])
        nc.sync.dma_start(hbTa[:HB, :], ins["zhbT"][:])
        nc.sync.dma_start(hbTa[HB:, :], ins["onesrowr"][:])
        nc.vector.memset(ring[:], 0.0)
        nc.vector.memset(dynstage[:], 0.0)

        # ---- conditioning network (one-time) ----
        ones1 = st.tile([1, P], f32, tag="ones1")
        nc.vector.memset(ones1[:], 1.0)
        tp = ps_t.tile([P, 512], f32, tag="tpw")
        nc.tensor.transpose(tp[:NF, 0:P], feat[:], ident[:])
        featTa = st.tile([NF + 1, P], f32, tag="featTa")
        nc.scalar.copy(featTa[:NF, :], tp[:NF, 0:P])
        nc.sync.dma_start(featTa[NF:, :], ins["onesrow"][:])

        h1ps = ps_l.tile([P, 2 * K], f32, tag="lps")
        nc.tensor.matmul(h1ps[:, :COND], featTa[:], wf1b[:], start=True, stop=True)
        h1 = wk.tile([P, COND], f32, tag="h1")
        nc.scalar.activation(h1[:], h1ps[:, :COND], AF.Tanh)

        tp = ps_t.tile([P, 512], f32, tag="tpw")
        nc.tensor.transpose(tp[:, 0:P], h1[:], ident[:])
        h1T = wk.tile([P, P], f32, tag="h1T")
        nc.scalar.copy(h1T[:], tp[:, 0:P])

        cps = ps_l.tile([P, 2 * K], f32, tag="lps")
        nc.tensor.matmul(cps[:, :COND], h1T[:], wf2[:], start=True, stop=False)
        nc.tensor.matmul(cps[:, :COND], ones1[:], bf2[:], start=False, stop=True)
        cond = wk.tile([P, COND], f32, tag="h1")
        nc.scalar.activation(cond[:], cps[:, :COND], AF.Tanh)

        tp = ps_t.tile([P, 512], f32, tag="tpw")
        nc.tensor.transpose(tp[:, 0:P], cond[:], ident[:])
        condT = wk.tile([P, P], f32, tag="h1T")
        nc.scalar.copy(condT[:], tp[:, 0:P])

        # gxs = cond @ Wxa[:COND] + ba  -> [P, 1152], then cast to bf16
        for sl in ((0, 512), (512, 1024), (1024, G3)):
            gsps = ps_l.tile([P, 2 * K], f32, tag="lps")
            nc.tensor.matmul(gsps[:, :sl[1] - sl[0]], condT[:], wxac[:, sl[0]:sl[1]],
                             start=True, stop=False)
            nc.tensor.matmul(gsps[:, :sl[1] - sl[0]], ones1[:], ba[:, sl[0]:sl[1]],
                             start=False, stop=True)
            nc.vector.tensor_copy(gxs[:, sl[0]:sl[1]], gsps[:, :sl[1] - sl[0]])
            nc.vector.tensor_copy(gxsr[:, sl[0]:sl[1]], gsps[:, :sl[1] - sl[0]])

        # ---- prologue: p(0) path + open accumulation groups for t=0 ----
        pdot = wk.tile([P, 1], f32, tag="pdot")
        sc16 = wk.tile([P, 16], f32, tag="sc16")
        nc.vector.scalar_tensor_tensor(sc16[:], lpcrot[:, 0:16], 0.0, ring[:],
                                       op0=ALU.bypass, op1=ALU.mult,
                                       accum_out=pdot[:])
        pscr = wk.tile([P, K], f32, tag="pscr")
        nc.vector.tensor_scalar(pscr[:], taub[:], pdot[:], 0.0,
                                op0=ALU.is_le, op1=ALU.add,
                                accum_out=dynstage[:, 0:1])

        def open_groups(dump=False):
            """Emit the t+1 accumulations that depend only on haT/hbTa/statics."""
            garz = ps_rz.tile([P, 768], f32, tag="garz")
            psn = ps_n.tile([P, HA + HB], f32, tag="ghn")
            ghn = psn[:, 0:HA]
            ghbn = psn[:, HA:HA + HB]
            gxn = ps_x.tile([P, HA], f32, tag="gxn")
            gb = ps_b.tile([P, 48], f32, tag="gb")
            # rz: gxs + sum_c haT_c @ Wha_c[rz]   (dyn closes later)
            # (single-matmul output is capped at one PSUM bank: 512 fp32)
            nc.tensor.matmul(garz[:, 0:512], identr[:], gxsr[:, 0:512],
                             start=True, stop=False)
            nc.tensor.matmul(garz[:, 512:768], identr[:], gxsr[:, 512:768],
                             start=True, stop=False)
            for c in range(3):
                w0 = c * G3
                nc.tensor.matmul(garz[:, 0:512], haT[c], wha[:, w0:w0 + 512],
                                 start=False, stop=False)
                nc.tensor.matmul(garz[:, 512:768], haT[c],
                                 wha[:, w0 + 512:w0 + 768],
                                 start=False, stop=False)
            # n (h-part): sum_c haT_c @ Wha_c[n]  (closed here)
            for c in range(3):
                w0 = c * G3
                nc.tensor.matmul(ghn[:], haT[c], wha[:, w0 + 768:w0 + G3],
                                 start=(c == 0), stop=(c == 2))
            # n (x-part): gxs_n  (dyn closes later)
            nc.tensor.matmul(gxn[:], identr[:], gxsr[:, 768:G3],
                             start=True, stop=False)
            # GRU-B: biases folded into the hbTa matmul (ones row of hbTa).
            # One accumulation group per PSUM bank: start=True clears the
            # whole bank's has_written bits, so ghb_n lives in the ps_n bank
            # (whose groups are emitted before it) and gb holds one group.
            nc.tensor.matmul(gb[:, 0:48], hbTa[:], whbbx[:, 0:48],
                             start=True, stop=False)
            nc.tensor.matmul(ghbn[:], hbTa[:], whbbx[:, 48:64],
                             start=True, stop=True)
            if dump:
                nc.sync.dma_start(dbg_outs["d_hbTa1"][:], hbTa[:])
                gbo_c = wk.tile([P, 64], f32, tag="dbg_gbo")
                nc.vector.tensor_copy(gbo_c[:, 0:48], gb[:])
                nc.vector.tensor_copy(gbo_c[:, 48:64], ghbn[:])
                nc.sync.dma_start(dbg_outs["d_gb1o"][:], gbo_c[:])
            return garz, ghn, ghbn, gxn, gb

        garz, ghn, ghbn, gxn, gb = open_groups()

        ha_rm = wk.tile([P, HA], f32, tag="ha_rm")
        nc.vector.memset(ha_rm[:], 0.0)
        hb_rm = wk.tile([P, HB], f32, tag="hb_rm")
        nc.vector.memset(hb_rm[:], 0.0)

        # ---- time loop ----
        for t in range(T_steps):
            # gumbel prefetch
            a2 = io.tile([P, K], f32, tag="a2")
            nc.sync.dma_start(a2[:], ins["gl"][t, :, :])

            # dyn features -> transposed [3, P]
            tpd = ps_t.tile([P, 512], f32, tag="tpw")
            nc.tensor.transpose(tpd[:3, 384:512], dynstage[:], ident[:])
            dynT = wk.tile([3, P], bf16, tag="dynT")
            nc.vector.tensor_copy(dynT[:], tpd[:3, 384:512])
            if dbg and t == 1:
                dynTf = wk.tile([3, P], f32, tag="dynTf")
                nc.vector.tensor_copy(dynTf[:], dynT[:])
                nc.sync.dma_start(dbg_outs["d_dynT1"][:], dynTf[:])

            # dyn matmuls close the rz / gxn groups (exact bf16 hi/lo)
            nc.tensor.matmul(garz[:, 0:512], dynT[:], wdynh[:, 0:512],
                             start=False, stop=False)
            nc.tensor.matmul(garz[:, 0:512], dynT[:], wdynl[:, 0:512],
                             start=False, stop=True)
            nc.tensor.matmul(garz[:, 512:768], dynT[:], wdynh[:, 512:768],
                             start=False, stop=False)
            nc.tensor.matmul(garz[:, 512:768], dynT[:], wdynl[:, 512:768],
                             start=False, stop=True)
            nc.tensor.matmul(gxn[:], dynT[:], wdynh[:, 768:G3],
                             start=False, stop=False)
            nc.tensor.matmul(gxn[:], dynT[:], wdynl[:, 768:G3],
                             start=False, stop=True)

            # GRU-A gates, tanh-only (sigmoid via th=tanh(x/2):
            # r*ghn == (th_r+1)*(0.5*ghn), 0.5 folded into Wha_n host-side;
            # z-blend: ha2 = an + z*(ha-an) == 0.5*((th_z+1)*(ha-an)) + an)
            thr = wk.tile([P, HA], f32, tag="thr")
            nc.scalar.activation(thr[:], garz[:, 0:HA], AF.Tanh, scale=0.5)
            thz = wk.tile([P, HA], f32, tag="thz")
            nc.scalar.activation(thz[:], garz[:, HA:768], AF.Tanh, scale=0.5)
            t1 = wk.tile([P, HA], f32, tag="t1")
            nc.vector.scalar_tensor_tensor(t1[:], thr[:], 1.0, ghn[:],
                                           op0=ALU.add, op1=ALU.mult)
            t3 = wk.tile([P, HA], f32, tag="t3")
            nc.vector.tensor_tensor(t3[:], t1[:], gxn[:], op=ALU.add)
            an = wk.tile([P, HA], f32, tag="an")
            nc.scalar.activation(an[:], t3[:], AF.Tanh)
            d = wk.tile([P, HA], f32, tag="d")
            nc.vector.tensor_tensor(d[:], ha_rm[:], an[:], op=ALU.subtract)
            zd = wk.tile([P, HA], f32, tag="zd")
            nc.vector.scalar_tensor_tensor(zd[:], thz[:], 1.0, d[:],
                                           op0=ALU.add, op1=ALU.mult)
            ha_rm = wk.tile([P, HA], f32, tag="ha_rm")
            nc.vector.scalar_tensor_tensor(ha_rm[:], zd[:], 0.5, an[:],
                                           op0=ALU.mult, op1=ALU.add)
            if dbg and t == 0:
                nc.sync.dma_start(dbg_outs["d_ha0"][:], ha_rm[:])
            if dbg and t == 1:
                garz_c = wk.tile([P, 768], f32, tag="dbg_garz")
                nc.vector.tensor_copy(garz_c[:], garz[:])
                nc.sync.dma_start(dbg_outs["d_garz1"][:], garz_c[:])
                ghn_c = wk.tile([P, HA], f32, tag="dbg_ghn")
                nc.vector.tensor_copy(ghn_c[:], ghn[:])
                nc.sync.dma_start(dbg_outs["d_ghn1"][:], ghn_c[:])
                gxn_c = wk.tile([P, HA], f32, tag="dbg_gxn")
                nc.vector.tensor_copy(gxn_c[:], gxn[:])
                nc.sync.dma_start(dbg_outs["d_gxn1"][:], gxn_c[:])
                nc.sync.dma_start(dbg_outs["d_r1"][:], r[:])
                nc.sync.dma_start(dbg_outs["d_an1"][:], an[:])
                nc.sync.dma_start(dbg_outs["d_ha1"][:], ha_rm[:])

            # ha2 -> haT, pipelined per chunk: transpose -> copy -> gxb MM
            tpc = ps_t.tile([P, 512], f32, tag="tpw")
            cp_eng = (nc.scalar.copy,
                      lambda o, i: nc.vector.tensor_copy(o, i),
                      nc.scalar.copy)
            for c in range(3):
                nc.tensor.transpose(tpc[:, c * P:(c + 1) * P],
                                    ha_rm[:, c * P:(c + 1) * P], ident[:])
                cp_eng[c](haTw[:, c * P:(c + 1) * P], tpc[:, c * P:(c + 1) * P])
                nc.tensor.matmul(gb[:, 0:48], haT[c], wxb[:, c * 48:(c + 1) * 48],
                                 start=False, stop=(c == 2))

            # GRU-B gates (same tanh-only scheme; 0.5 folded into whbbx_n)
            thb = wk.tile([P, 2 * HB], f32, tag="thb")
            nc.scalar.activation(thb[:], gb[:, 0:32], AF.Tanh, scale=0.5)
            t1b = wk.tile([P, HB], f32, tag="t1b")
            nc.vector.scalar_tensor_tensor(t1b[:], thb[:, 0:HB], 1.0, ghbn[:],
                                           op0=ALU.add, op1=ALU.mult)
            t2b = wk.tile([P, HB], f32, tag="t2b")
            nc.vector.tensor_tensor(t2b[:], t1b[:], gb[:, 32:48], op=ALU.add)
            nb = wk.tile([P, HB], f32, tag="nb")
            nc.scalar.activation(nb[:], t2b[:], AF.Tanh)
            warm = ps_w.tile([P, 64], f32, tag="warm")
            nc.tensor.matmul(warm[:, 0:16], ones1[:], nb[0:1, 0:16], start=True,
                             stop=True)
            db = wk.tile([P, HB], f32, tag="db")
            nc.vector.tensor_tensor(db[:], hb_rm[:], nb[:], op=ALU.subtract)
            zdb = wk.tile([P, HB], f32, tag="zdb")
            nc.vector.scalar_tensor_tensor(zdb[:], thb[:, HB:2 * HB], 1.0, db[:],
                                           op0=ALU.add, op1=ALU.mult)
            hb_rm = wk.tile([P, HB], f32, tag="hb_rm")
            nc.vector.scalar_tensor_tensor(hb_rm[:], zdb[:], 0.5, nb[:],
                                           op0=ALU.mult, op1=ALU.add)
            if dbg and t == 0:
                nc.sync.dma_start(dbg_outs["d_hb0"][:], hb_rm[:])
            if dbg and t == 1:
                gb_c = wk.tile([P, 64], f32, tag="dbg_gb")
                nc.vector.tensor_copy(gb_c[:, 0:48], gb[:])
                nc.vector.tensor_copy(gb_c[:, 48:64], ghbn[:])
                nc.sync.dma_start(dbg_outs["d_gb1"][:], gb_c[:])
                nc.sync.dma_start(dbg_outs["d_hb1"][:], hb_rm[:])

            # hb -> hbT, logits
            tpb = ps_t.tile([P, 512], f32, tag="tpw")
            nc.tensor.transpose(tpb[:HB, 384:512], hb_rm[:], ident[:])
            nc.scalar.copy(hbTa[:HB, :], tpb[:HB, 384:512])

            lps = ps_l.tile([P, 2 * K], f32, tag="lps")
            nc.tensor.matmul(lps[:], hbTa[:], w12b[:], start=True, stop=True)
            l12 = wk.tile([P, 2 * K], f32, tag="l12")
            nc.scalar.activation(l12[:], lps[:], AF.Tanh)
            lg12 = wk.tile([P, 2 * K], f32, tag="lg12")
            nc.vector.tensor_tensor(lg12[:, 0:K], l12[:, 0:K], g12[:, 0:K],
                                    op=ALU.mult)
            nc.gpsimd.tensor_tensor(lg12[:, K:2 * K], l12[:, K:2 * K],
                                    g12[:, K:2 * K], op=ALU.mult)
            logits = io.tile([P, K], f32, tag="logits")
            nc.vector.tensor_tensor(logits[:, 0:128], lg12[:, 0:128],
                                    lg12[:, K:K + 128], op=ALU.add)
            nc.gpsimd.tensor_tensor(logits[:, 128:K], lg12[:, 128:K],
                                    lg12[:, K + 128:2 * K], op=ALU.add)
            nc.sync.dma_start(out_l[t, :, :], logits[:])

            # gumbel softmax expected index
            zz = wk.tile([P, K], f32, tag="zz")
            nc.vector.tensor_tensor(zz[:, 0:128], logits[:, 0:128], a2[:, 0:128],
                                    op=ALU.subtract)
            nc.gpsimd.tensor_tensor(zz[:, 128:K], logits[:, 128:K], a2[:, 128:K],
                                    op=ALU.subtract)
            E = wk.tile([P, K], f32, tag="E")
            den = wk.tile([P, 1], f32, tag="den")
            nc.scalar.activation(E[:], zz[:], AF.Exp, accum_out=den[:])
            warm = ps_w.tile([P, 64], f32, tag="warm")
            nc.tensor.matmul(warm[:], ones1[:], E[0:1, 0:64], start=True,
                             stop=True)
            Escr = wk.tile([P, K], f32, tag="Escr")
            num = wk.tile([P, 1], f32, tag="num")
            nc.vector.scalar_tensor_tensor(Escr[:], E[:], 0.0, idxb[:],
                                           op0=ALU.bypass, op1=ALU.mult,
                                           accum_out=num[:])
            rden = wk.tile([P, 1], f32, tag="rden")
            nc.vector.reciprocal(rden[:], den[:])
            soft = wk.tile([P, 1], f32, tag="soft")
            nc.vector.tensor_tensor(soft[:], num[:], rden[:], op=ALU.mult)
            if dbg and t == 0:
                nc.sync.dma_start(dbg_outs["d_soft0"][:], soft[:])
            # e = round-half-even(soft) == jnp.round (verified on HW)
            eint = wk.tile([P, 1], i32, tag="eint")
            nc.vector.tensor_copy(eint[:], soft[:])
            nc.vector.tensor_copy(dynstage[:, 2:3], eint[:])
            # s = p + e -> ring slot; col1 = p (before tau overwrites col0)
            nc.vector.tensor_tensor(ring[:, t % 16:t % 16 + 1], dynstage[:, 0:1],
                                    dynstage[:, 2:3], op=ALU.add)
            nc.vector.tensor_copy(dynstage[:, 1:2], dynstage[:, 0:1])
            # p(t+1) path
            rot = ((t + 1) % 16) * 16
            sc16 = wk.tile([P, 16], f32, tag="sc16")
            pdot = wk.tile([P, 1], f32, tag="pdot")
            nc.vector.scalar_tensor_tensor(sc16[:], lpcrot[:, rot:rot + 16], 0.0,
                                           ring[:], op0=ALU.bypass, op1=ALU.mult,
                                           accum_out=pdot[:])
            pscr = wk.tile([P, K], f32, tag="pscr")
            nc.vector.tensor_scalar(pscr[:], taub[:], pdot[:], 0.0,
                                    op0=ALU.is_le, op1=ALU.add,
                                    accum_out=dynstage[:, 0:1])
            warm = ps_w.tile([P, 64], f32, tag="warm")
            nc.tensor.matmul(warm[:], ones1[:], pscr[0:1, 0:64], start=True,
                             stop=True)
            if dbg and t == 0:
                nc.sync.dma_start(dbg_outs["d_e0"][:], dynstage[:, 2:3])
                nc.sync.dma_start(dbg_outs["d_s0"][:], ring[:, 0:1])
                nc.sync.dma_start(dbg_outs["d_p0"][:], dynstage[:, 1:2])
                nc.sync.dma_start(dbg_outs["d_p1"][:], dynstage[:, 0:1])
                nc.sync.dma_start(dbg_outs["d_pdot1"][:], pdot[:])

            # open accumulation groups for t+1 (hidden under this step's tail)
            if t + 1 < T_steps:
                garz, ghn, ghbn, gxn, gb = open_groups(dump=(dbg and t == 0))

    return nc, ins, out_l


def _pack_inputs(frames_features, lpc_coeffs, gumbel_u, Wf1, bf1, Wf2, bf2,
                 Wxa, Wha, ba, Wxb, Whb, bb, W1, b1, g1, W2, b2, g2,
                 T_steps=T):
    """Host-side packing -> list of per-core input dicts."""
    import ml_dtypes
    fp = np.float32
    bf = ml_dtypes.bfloat16
    feat = np.ascontiguousarray(frames_features, fp).reshape(R, NF)
    lpc = np.ascontiguousarray(lpc_coeffs, fp).reshape(R, M)
    u = np.ascontiguousarray(gumbel_u, fp)
    gl = np.log(-np.log(u[:T_steps])).astype(fp)

    # lpcrot[:, 16*r + j] = lpc[:, (j - r) % 16]
    lpcrot = np.empty((R, 256), fp)
    for r_ in range(16):
        for j in range(16):
            lpcrot[:, 16 * r_ + j] = lpc[:, (j - r_) % 16]

    wha_s = np.asarray(Wha, fp).copy()
    wha_s[:, 2 * 384:] *= fp(0.5)        # 0.5*gh_n for the tanh-only r-gate
    wha_p = np.concatenate([np.ascontiguousarray(wha_s[c * P:(c + 1) * P, :])
                            for c in range(3)], axis=1)          # [128, 3*1152]
    # wxb chunk c: [rz 32 | n 16]
    wxb_p = np.concatenate(
        [np.concatenate([np.asarray(Wxb, fp)[c * P:(c + 1) * P, 0:32],
                         np.asarray(Wxb, fp)[c * P:(c + 1) * P, 32:48]], axis=1)
         for c in range(3)], axis=1)                             # [128, 144]
    statics = {
        "wf1b": np.concatenate([np.asarray(Wf1, fp), np.asarray(bf1, fp)[None, :]], 0),
        "wf2": np.asarray(Wf2, fp),
        "bf2": np.asarray(bf2, fp)[None, :],
        "wxac": np.ascontiguousarray(np.asarray(Wxa, fp)[:COND, :]),
        "ba": np.asarray(ba, fp)[None, :],
        "wdynh": _wdyn2(np.asarray(Wxa, fp))[0:3].astype(bf),
        "wdynl": _wdyn2(np.asarray(Wxa, fp))[3:6].astype(bf),
        "wha": wha_p,
        "wxb": wxb_p,
        "whbbx": np.concatenate([
            np.concatenate([np.asarray(Whb, fp)[:, 0:32],
                            np.asarray(bb, fp)[None, 0:32]], 0),
            np.concatenate([np.zeros((HB, HB), fp),
                            np.asarray(bb, fp)[None, 32:48]], 0),
            np.concatenate([np.asarray(Whb, fp)[:, 32:48] * fp(0.5),
                            np.zeros((1, HB), fp)], 0)], axis=1),
        "w12b": np.concatenate([
            np.concatenate([np.asarray(W1, fp), np.asarray(W2, fp)], axis=1),
            np.concatenate([np.asarray(b1, fp), np.asarray(b2, fp)])[None, :]],
            0),
        "g12": np.repeat(np.concatenate([np.asarray(g1, fp), np.asarray(g2, fp)])[None, :], P, 0),
        "idxb": np.repeat(np.arange(K, dtype=fp)[None, :], P, 0),
        "ident": np.eye(P, dtype=fp),
        "identr": np.eye(P, dtype=fp),
        "onesrow": np.ones((1, P), fp),
        "onesrowr": np.ones((1, P), fp),
        "zhaT": np.zeros((P, 3 * P), fp),
        "zhbT": np.zeros((HB, P), fp),
        "taub": np.repeat(_tau_table()[None, :], P, 0),
    }
    per_core = []
    for c in range(N_CORES):
        rs = slice(c * P, (c + 1) * P)
        m = dict(statics)
        m["feat"] = np.ascontiguousarray(feat[rs])
        m["lpcrot"] = np.ascontiguousarray(lpcrot[rs])
        m["gl"] = np.ascontiguousarray(gl[:, rs, :])
        per_core.append(m)
    return per_core


def _wdyn2(Wxa):
    """[6, 3H]: bf16 hi/lo split of [w_p, w_s, w_s+w_e] (exact-bf16 dyn matmul)."""
    import ml_dtypes
    fp = np.float32
    wd = Wxa[COND:COND + 3, :].astype(fp).copy()
    wd[2] = (wd[1] + wd[2]).astype(fp)
    hi = wd.astype(ml_dtypes.bfloat16).astype(fp)
    lo = (wd - hi).astype(ml_dtypes.bfloat16).astype(fp)
    return np.concatenate([hi, lo], 0)


def _tau_table():
    """tau[k] = smallest float32 x with mu_law_p(x) >= k+1 (k=0..254);
    tau[255] = +inf sentinel. p(x) = sum_k [x >= tau_k]."""
    fp = np.float32

    def p_of(x):
        x = np.asarray(x, fp)
        xc = np.clip(x, fp(-1.0), fp(1.0)).astype(fp)
        ln_mu1 = np.log(fp(256.0)).astype(fp)
        y = (np.sign(xc) * np.log1p(fp(255.0) * np.abs(xc)) / ln_mu1).astype(fp)
        v = ((y + fp(1.0)) * fp(0.5) * fp(256.0)).astype(fp)
        return np.clip(np.floor(v), 0.0, 255.0)

    def f2i(x):
        b = np.asarray(x, np.float32).view(np.int32)
        return np.where(b < 0, np.int32(-2147483648) - b, b).astype(np.int64)

    def i2f(i):
        i = np.asarray(i, np.int64)
        b = np.where(i < 0, -2147483648 - i, i).astype(np.int32)
        return b.view(np.float32)

    ks = np.arange(1, 256)
    lo = np.full(255, f2i(np.float32(-1.5)), np.int64)
    hi = np.full(255, f2i(np.float32(1.5)), np.int64)
    for _ in range(40):
        mid = (lo + hi) // 2
        ge = p_of(i2f(mid)) >= ks
        hi = np.where(ge, mid, hi)
        lo = np.where(ge, lo, mid)
    tau = i2f(hi).astype(fp)
    out = np.empty(256, fp)
    out[:255] = tau
    out[255] = np.float32(3.0e38)
    return out


_CACHE = {}


def _ensure_devices():
    import jax
    try:
        if len(jax.devices()) >= N_CORES:
            return
    except Exception:
        pass
    jax.config.update("jax_platforms", "axon,cpu")
    import jax.extend.backend as _jeb
    _jeb.clear_backends()
    assert len(jax.devices()) >= N_CORES, (
        f"need {N_CORES} NeuronCores, visible: {jax.devices()}")


def _get_nc(T_steps):
    if T_steps not in _CACHE:
        nc, ins, out_l = _build(T_steps)
        nc.compile()
        _CACHE[T_steps] = nc
    return _CACHE[T_steps]


def kernel(**inputs):
    _ensure_devices()
    nc = _get_nc(T)
    per_core = _pack_inputs(**inputs)
    res = run_bass_kernel_spmd(nc, per_core, list(range(N_CORES)))
    shards = [res.results[c]["logits"] for c in range(N_CORES)]   # each [T,128,K]
    logits_seq = np.concatenate(shards, axis=1)                   # [T, R, K]
    out = logits_seq.transpose(1, 0, 2).reshape(B, F * T, K)
    return np.ascontiguousarray(out, dtype=np.float32)


# revision 25
# speedup vs baseline: 1.0346x; 1.0346x over previous
"""LPCNet sampling kernel for Trainium2 — nn_LPCNet_91061896609827.

kernel(**inputs) takes FULL unsharded inputs (as from reference.setup_inputs())
and returns the FULL [B, F*T, K] float32 logits output.

Strategy: data-parallel over the R = B*F = 1024 row axis, 8 shards of 128 rows
(one per NeuronCore, rows on SBUF partitions), GRU/dense weights replicated.
The T=160 sequential sampling scan runs fully on-device per core.

v2 vs v1 (5.01ms):
  - gumbel g = ln(-ln u) precomputed on HOST; no device prepass, no u DMA.
  - all recurrent matmuls single-pass bf16 (fp32 matmul = 2 PE passes);
    dyn (p/s/e feature) matmul keeps exact bf16 hi/lo split.
  - static gate biases (cond@Wxa+ba, bb) folded into PSUM accumulation via
    identity/ones matmuls -> sigmoid/tanh read PSUM directly, the wide
    vector adds disappear from the serial chain.
  - garz accumulation reordered: the big wha matmuls for step t+1 are
    emitted at the end of step t (hidden under GRU-B/sampling); only the
    tiny dyn matmuls sit on the critical path.
  - round(soft) via direct f32->i32 cast (hardware round-half-even ==
    jnp.round; verified by probe).
  - sigmoid LUT (one op) instead of tanh(0.5x) rescaling (three ops);
    r/z sigmoids split so r is ready earlier.
  - ha transpose copies spread across scalar/vector/gpsimd engines.

Self-contained: hardcodes shapes; reads nothing from /root/problem.
"""
import numpy as np
from contextlib import ExitStack

import concourse.bass as bass
import concourse.tile as tile
import concourse.mybir as mybir
from concourse import bacc
from concourse.bass_utils import run_bass_kernel_spmd

B, F, M, NF = 32, 32, 16, 20
T, K = 160, 256
R = B * F
COND, HA, HB = 128, 384, 16
N_CORES = 8
P = 128  # rows per core == SBUF partitions

f32 = mybir.dt.float32
bf16 = mybir.dt.bfloat16
i32 = mybir.dt.int32
f32r = mybir.dt.float32r
AF = mybir.ActivationFunctionType
ALU = mybir.AluOpType

G3 = 3 * HA  # 1152


def _build(T_steps: int, dbg: bool = False):
    nc = bacc.Bacc("TRN2", target_bir_lowering=False, debug=False,
                   num_devices=N_CORES)

    def din(name, shape, dt=f32):
        return nc.dram_tensor(name, list(shape), dt, kind="ExternalInput").ap()

    dbg_outs = {}
    if dbg:
        for nm, sh in [("d_soft0", [P, 1]), ("d_e0", [P, 1]), ("d_s0", [P, 1]),
                       ("d_p0", [P, 1]), ("d_p1", [P, 1]), ("d_pdot1", [P, 1]),
                       ("d_ha0", [P, HA]), ("d_hb0", [P, HB]),
                       ("d_garz1", [P, 768]), ("d_ghn1", [P, HA]),
                       ("d_gxn1", [P, HA]), ("d_r1", [P, HA]),
                       ("d_an1", [P, HA]), ("d_ha1", [P, HA]),
                       ("d_gb1", [P, 64]), ("d_hb1", [P, HB]),
                       ("d_dynT1", [3, P]), ("d_hbTa1", [HB + 1, P]),
                       ("d_gb1o", [P, 64])]:
            dbg_outs[nm] = nc.dram_tensor(nm, sh, f32, kind="ExternalOutput").ap()

    ins = {
        # per-core
        "feat": din("feat", [P, NF]),
        "lpcrot": din("lpcrot", [P, 16 * 16]),
        "gl": din("gl", [T_steps, P, K]),          # ln(-ln u), host-computed
        # replicated statics (host-packed)
        "wf1b": din("wf1b", [NF + 1, COND]),
        "wf2": din("wf2", [COND, COND]),
        "bf2": din("bf2", [1, COND]),
        "wxac": din("wxac", [COND, G3]),
        "ba": din("ba", [1, G3]),
        "wdynh": din("wdynh", [3, G3], bf16),
        "wdynl": din("wdynl", [3, G3], bf16),
        "wha": din("wha", [P, 3 * G3], f32r),      # 3 K-chunks on free axis
        "wxb": din("wxb", [P, 3 * 48], f32r),      # chunk c: [rz 32 | n 16]
        "whbbx": din("whbbx", [HB + 1, 64], f32r),  # [[Whb_rz;bb_rz] | [0;bb_n] | [Whb_n;0]]
        "w12b": din("w12b", [HB + 1, 2 * K], f32r),  # [W1|W2 ; b1|b2]
        "g12": din("g12", [P, 2 * K]),             # [g1|g2] row broadcast
        "idxb": din("idxb", [P, K]),               # iota row broadcast
        "ident": din("ident", [P, P]),
        "identr": din("identr", [P, P], f32r),
        "onesrow": din("onesrow", [1, P]),
        "onesrowr": din("onesrowr", [1, P], f32r),
        "zhaT": din("zhaT", [P, 3 * P], f32r),
        "zhbT": din("zhbT", [HB, P], f32r),
        "taub": din("taub", [P, K]),
    }
    out_l = nc.dram_tensor("logits", [T_steps, P, K], f32,
                           kind="ExternalOutput").ap()

    with tile.TileContext(nc) as tc, ExitStack() as ctx:
        st = ctx.enter_context(tc.tile_pool(name="static", bufs=1))
        wk = ctx.enter_context(tc.tile_pool(name="work", bufs=2))
        io = ctx.enter_context(tc.tile_pool(name="io", bufs=3))
        ps_rz = ctx.enter_context(tc.tile_pool(name="ps_rz", bufs=1, space="PSUM"))
        ps_n = ctx.enter_context(tc.tile_pool(name="ps_n", bufs=1, space="PSUM"))
        ps_x = ctx.enter_context(tc.tile_pool(name="ps_x", bufs=1, space="PSUM"))
        ps_b = ctx.enter_context(tc.tile_pool(name="ps_b", bufs=1, space="PSUM"))
        ps_l = ctx.enter_context(tc.tile_pool(name="ps_l", bufs=1, space="PSUM"))
        ps_t = ctx.enter_context(tc.tile_pool(name="ps_t", bufs=1, space="PSUM"))
        ps_w = ctx.enter_context(tc.tile_pool(name="ps_w", bufs=1, space="PSUM"))

        # ---- load statics ----
        def load(name, shape, dt=f32, tag=None):
            t_ = st.tile(list(shape), dt, tag=tag or name)
            nc.sync.dma_start(t_[:], ins[name][:])
            return t_

        feat = load("feat", [P, NF])
        lpcrot = load("lpcrot", [P, 256])
        wf1b = load("wf1b", [NF + 1, COND])
        wf2 = load("wf2", [COND, COND])
        bf2 = load("bf2", [1, COND])
        wxac = load("wxac", [COND, G3])
        ba = load("ba", [1, G3])
        wdynh = load("wdynh", [3, G3], bf16)
        wdynl = load("wdynl", [3, G3], bf16)
        wha = load("wha", [P, 3 * G3], f32r)
        wxb = load("wxb", [P, 3 * 48], f32r)
        whbbx = load("whbbx", [HB + 1, 64], f32r)
        w12b = load("w12b", [HB + 1, 2 * K], f32r)
        g12 = load("g12", [P, 2 * K])
        idxb = load("idxb", [P, K])
        ident = load("ident", [P, P])
        identr = load("identr", [P, P], f32r)
        taub = load("taub", [P, K])

        # ---- persistent state ----
        haTw = st.tile([P, 3 * P], f32r, tag="haTw")
        haT = [haTw[:, c * P:(c + 1) * P] for c in range(3)]
        hbTa = st.tile([HB + 1, P], f32r, tag="hbTa")
        ring = st.tile([P, 16], f32, tag="ring")
        dynstage = st.tile([P, 3], f32, tag="dynstage")
        gxs = st.tile([P, G3], f32, tag="gxs")
        gxsr = st.tile([P, G3], f32r, tag="gxsr")

        nc.sync.dma_start(haTw[:], ins["zhaT"][:])
        nc.sync.dma_start(hbTa[:HB, :], ins["zhbT"][:])
        nc.sync.dma_start(hbTa[HB:, :], ins["onesrowr"][:])
        nc.vector.memset(ring[:], 0.0)
        nc.vector.memset(dynstage[:], 0.0)

        # ---- conditioning network (one-time) ----
        ones1 = st.tile([1, P], f32, tag="ones1")
        nc.vector.memset(ones1[:], 1.0)
        tp = ps_t.tile([P, 512], f32, tag="tpw")
        nc.tensor.transpose(tp[:NF, 0:P], feat[:], ident[:])
        featTa = st.tile([NF + 1, P], f32, tag="featTa")
        nc.scalar.copy(featTa[:NF, :], tp[:NF, 0:P])
        nc.sync.dma_start(featTa[NF:, :], ins["onesrow"][:])

        h1ps = ps_l.tile([P, 2 * K], f32, tag="lps")
        nc.tensor.matmul(h1ps[:, :COND], featTa[:], wf1b[:], start=True, stop=True)
        h1 = wk.tile([P, COND], f32, tag="h1")
        nc.scalar.activation(h1[:], h1ps[:, :COND], AF.Tanh)

        tp = ps_t.tile([P, 512], f32, tag="tpw")
        nc.tensor.transpose(tp[:, 0:P], h1[:], ident[:])
        h1T = wk.tile([P, P], f32, tag="h1T")
        nc.scalar.copy(h1T[:], tp[:, 0:P])

        cps = ps_l.tile([P, 2 * K], f32, tag="lps")
        nc.tensor.matmul(cps[:, :COND], h1T[:], wf2[:], start=True, stop=False)
        nc.tensor.matmul(cps[:, :COND], ones1[:], bf2[:], start=False, stop=True)
        cond = wk.tile([P, COND], f32, tag="h1")
        nc.scalar.activation(cond[:], cps[:, :COND], AF.Tanh)

        tp = ps_t.tile([P, 512], f32, tag="tpw")
        nc.tensor.transpose(tp[:, 0:P], cond[:], ident[:])
        condT = wk.tile([P, P], f32, tag="h1T")
        nc.scalar.copy(condT[:], tp[:, 0:P])

        # gxs = cond @ Wxa[:COND] + ba  -> [P, 1152], then cast to bf16
        for sl in ((0, 512), (512, 1024), (1024, G3)):
            gsps = ps_l.tile([P, 2 * K], f32, tag="lps")
            nc.tensor.matmul(gsps[:, :sl[1] - sl[0]], condT[:], wxac[:, sl[0]:sl[1]],
                             start=True, stop=False)
            nc.tensor.matmul(gsps[:, :sl[1] - sl[0]], ones1[:], ba[:, sl[0]:sl[1]],
                             start=False, stop=True)
            nc.vector.tensor_copy(gxs[:, sl[0]:sl[1]], gsps[:, :sl[1] - sl[0]])
            nc.vector.tensor_copy(gxsr[:, sl[0]:sl[1]], gsps[:, :sl[1] - sl[0]])

        # ---- prologue: p(0) path + open accumulation groups for t=0 ----
        pdot = wk.tile([P, 1], f32, tag="pdot")
        sc16 = wk.tile([P, 16], f32, tag="sc16")
        nc.vector.scalar_tensor_tensor(sc16[:], lpcrot[:, 0:16], 0.0, ring[:],
                                       op0=ALU.bypass, op1=ALU.mult,
                                       accum_out=pdot[:])
        pscr = wk.tile([P, K], f32, tag="pscr")
        nc.vector.tensor_scalar(pscr[:], taub[:], pdot[:], 0.0,
                                op0=ALU.is_le, op1=ALU.add,
                                accum_out=dynstage[:, 0:1])

        def open_groups(dump=False):
            """Emit the t+1 accumulations that depend only on haT/hbTa/statics."""
            garz = ps_rz.tile([P, 768], f32, tag="garz")
            psn = ps_n.tile([P, HA + HB], f32, tag="ghn")
            ghn = psn[:, 0:HA]
            ghbn = psn[:, HA:HA + HB]
            gxn = ps_x.tile([P, HA], f32, tag="gxn")
            gb = ps_b.tile([P, 48], f32, tag="gb")
            # rz: gxs + sum_c haT_c @ Wha_c[rz]   (dyn closes later)
            # (single-matmul output is capped at one PSUM bank: 512 fp32)
            nc.tensor.matmul(garz[:, 0:512], identr[:], gxsr[:, 0:512],
                             start=True, stop=False)
            nc.tensor.matmul(garz[:, 512:768], identr[:], gxsr[:, 512:768],
                             start=True, stop=False)
            for c in range(3):
                w0 = c * G3
                nc.tensor.matmul(garz[:, 0:512], haT[c], wha[:, w0:w0 + 512],
                                 start=False, stop=False)
                nc.tensor.matmul(garz[:, 512:768], haT[c],
                                 wha[:, w0 + 512:w0 + 768],
                                 start=False, stop=False)
            # n (h-part): sum_c haT_c @ Wha_c[n]  (closed here)
            for c in range(3):
                w0 = c * G3
                nc.tensor.matmul(ghn[:], haT[c], wha[:, w0 + 768:w0 + G3],
                                 start=(c == 0), stop=(c == 2))
            # n (x-part): gxs_n  (dyn closes later)
            nc.tensor.matmul(gxn[:], identr[:], gxsr[:, 768:G3],
                             start=True, stop=False)
            # GRU-B: biases folded into the hbTa matmul (ones row of hbTa).
            # One accumulation group per PSUM bank: start=True clears the
            # whole bank's has_written bits, so ghb_n lives in the ps_n bank
            # (whose groups are emitted before it) and gb holds one group.
            nc.tensor.matmul(gb[:, 0:48], hbTa[:], whbbx[:, 0:48],
                             start=True, stop=False)
            nc.tensor.matmul(ghbn[:], hbTa[:], whbbx[:, 48:64],
                             start=True, stop=True)
            if dump:
                nc.sync.dma_start(dbg_outs["d_hbTa1"][:], hbTa[:])
                gbo_c = wk.tile([P, 64], f32, tag="dbg_gbo")
                nc.vector.tensor_copy(gbo_c[:, 0:48], gb[:])
                nc.vector.tensor_copy(gbo_c[:, 48:64], ghbn[:])
                nc.sync.dma_start(dbg_outs["d_gb1o"][:], gbo_c[:])
            return garz, ghn, ghbn, gxn, gb

        garz, ghn, ghbn, gxn, gb = open_groups()

        ha_rm = wk.tile([P, HA], f32, tag="ha_rm")
        nc.vector.memset(ha_rm[:], 0.0)
        hb_rm = wk.tile([P, HB], f32, tag="hb_rm")
        nc.vector.memset(hb_rm[:], 0.0)

        # ---- time loop ----
        for t in range(T_steps):
            # gumbel prefetch
            a2 = io.tile([P, K], f32, tag="a2")
            nc.sync.dma_start(a2[:], ins["gl"][t, :, :])

            # dyn features -> transposed [3, P]
            tpd = ps_t.tile([P, 512], f32, tag="tpw")
            nc.tensor.transpose(tpd[:3, 384:512], dynstage[:], ident[:])
            dynT = wk.tile([3, P], bf16, tag="dynT")
            nc.vector.tensor_copy(dynT[:], tpd[:3, 384:512])
            if dbg and t == 1:
                dynTf = wk.tile([3, P], f32, tag="dynTf")
                nc.vector.tensor_copy(dynTf[:], dynT[:])
                nc.sync.dma_start(dbg_outs["d_dynT1"][:], dynTf[:])

            # dyn matmuls close the rz / gxn groups (exact bf16 hi/lo)
            nc.tensor.matmul(garz[:, 0:512], dynT[:], wdynh[:, 0:512],
                             start=False, stop=False)
            nc.tensor.matmul(garz[:, 0:512], dynT[:], wdynl[:, 0:512],
                             start=False, stop=True)
            nc.tensor.matmul(garz[:, 512:768], dynT[:], wdynh[:, 512:768],
                             start=False, stop=False)
            nc.tensor.matmul(garz[:, 512:768], dynT[:], wdynl[:, 512:768],
                             start=False, stop=True)
            nc.tensor.matmul(gxn[:], dynT[:], wdynh[:, 768:G3],
                             start=False, stop=False)
            nc.tensor.matmul(gxn[:], dynT[:], wdynl[:, 768:G3],
                             start=False, stop=True)

            # GRU-A gates, tanh-only (sigmoid via th=tanh(x/2):
            # r*ghn == (th_r+1)*(0.5*ghn), 0.5 folded into Wha_n host-side;
            # z-blend: ha2 = an + z*(ha-an) == 0.5*((th_z+1)*(ha-an)) + an)
            thr = wk.tile([P, HA], f32, tag="thr")
            nc.scalar.activation(thr[:], garz[:, 0:HA], AF.Tanh, scale=0.5)
            thz = wk.tile([P, HA], f32, tag="thz")
            nc.scalar.activation(thz[:], garz[:, HA:768], AF.Tanh, scale=0.5)
            t1 = wk.tile([P, HA], f32, tag="t1")
            nc.vector.scalar_tensor_tensor(t1[:], thr[:], 1.0, ghn[:],
                                           op0=ALU.add, op1=ALU.mult)
            t3 = wk.tile([P, HA], f32, tag="t3")
            nc.vector.tensor_tensor(t3[:], t1[:], gxn[:], op=ALU.add)
            an = wk.tile([P, HA], f32, tag="an")
            nc.scalar.activation(an[:], t3[:], AF.Tanh)
            d = wk.tile([P, HA], f32, tag="d")
            nc.vector.tensor_tensor(d[:], ha_rm[:], an[:], op=ALU.subtract)
            zd = wk.tile([P, HA], f32, tag="zd")
            nc.vector.scalar_tensor_tensor(zd[:], thz[:], 1.0, d[:],
                                           op0=ALU.add, op1=ALU.mult)
            ha_rm = wk.tile([P, HA], f32, tag="ha_rm")
            nc.vector.scalar_tensor_tensor(ha_rm[:], zd[:], 0.5, an[:],
                                           op0=ALU.mult, op1=ALU.add)
            if dbg and t == 0:
                nc.sync.dma_start(dbg_outs["d_ha0"][:], ha_rm[:])
            if dbg and t == 1:
                garz_c = wk.tile([P, 768], f32, tag="dbg_garz")
                nc.vector.tensor_copy(garz_c[:], garz[:])
                nc.sync.dma_start(dbg_outs["d_garz1"][:], garz_c[:])
                ghn_c = wk.tile([P, HA], f32, tag="dbg_ghn")
                nc.vector.tensor_copy(ghn_c[:], ghn[:])
                nc.sync.dma_start(dbg_outs["d_ghn1"][:], ghn_c[:])
                gxn_c = wk.tile([P, HA], f32, tag="dbg_gxn")
                nc.vector.tensor_copy(gxn_c[:], gxn[:])
                nc.sync.dma_start(dbg_outs["d_gxn1"][:], gxn_c[:])
                nc.sync.dma_start(dbg_outs["d_r1"][:], r[:])
                nc.sync.dma_start(dbg_outs["d_an1"][:], an[:])
                nc.sync.dma_start(dbg_outs["d_ha1"][:], ha_rm[:])

            # ha2 -> haT: transposes back-to-back on PE, copies on
            # scalar/vector, then the gxb matmuls
            tpc = ps_t.tile([P, 512], f32, tag="tpw")
            for c in range(3):
                nc.tensor.transpose(tpc[:, c * P:(c + 1) * P],
                                    ha_rm[:, c * P:(c + 1) * P], ident[:])
            nc.scalar.copy(haTw[:, 0:P], tpc[:, 0:P])
            nc.vector.tensor_copy(haTw[:, P:2 * P], tpc[:, P:2 * P])
            nc.scalar.copy(haTw[:, 2 * P:3 * P], tpc[:, 2 * P:3 * P])
            for c in range(3):
                nc.tensor.matmul(gb[:, 0:48], haT[c], wxb[:, c * 48:(c + 1) * 48],
                                 start=False, stop=(c == 2))

            # GRU-B gates (same tanh-only scheme; 0.5 folded into whbbx_n)
            thb = wk.tile([P, 2 * HB], f32, tag="thb")
            nc.scalar.activation(thb[:], gb[:, 0:32], AF.Tanh, scale=0.5)
            t1b = wk.tile([P, HB], f32, tag="t1b")
            nc.vector.scalar_tensor_tensor(t1b[:], thb[:, 0:HB], 1.0, ghbn[:],
                                           op0=ALU.add, op1=ALU.mult)
            t2b = wk.tile([P, HB], f32, tag="t2b")
            nc.vector.tensor_tensor(t2b[:], t1b[:], gb[:, 32:48], op=ALU.add)
            nb = wk.tile([P, HB], f32, tag="nb")
            nc.scalar.activation(nb[:], t2b[:], AF.Tanh)
            warm = ps_w.tile([P, 64], f32, tag="warm")
            nc.tensor.matmul(warm[:, 0:16], ones1[:], nb[0:1, 0:16], start=True,
                             stop=True)
            db = wk.tile([P, HB], f32, tag="db")
            nc.vector.tensor_tensor(db[:], hb_rm[:], nb[:], op=ALU.subtract)
            zdb = wk.tile([P, HB], f32, tag="zdb")
            nc.vector.scalar_tensor_tensor(zdb[:], thb[:, HB:2 * HB], 1.0, db[:],
                                           op0=ALU.add, op1=ALU.mult)
            hb_rm = wk.tile([P, HB], f32, tag="hb_rm")
            nc.vector.scalar_tensor_tensor(hb_rm[:], zdb[:], 0.5, nb[:],
                                           op0=ALU.mult, op1=ALU.add)
            if dbg and t == 0:
                nc.sync.dma_start(dbg_outs["d_hb0"][:], hb_rm[:])
            if dbg and t == 1:
                gb_c = wk.tile([P, 64], f32, tag="dbg_gb")
                nc.vector.tensor_copy(gb_c[:, 0:48], gb[:])
                nc.vector.tensor_copy(gb_c[:, 48:64], ghbn[:])
                nc.sync.dma_start(dbg_outs["d_gb1"][:], gb_c[:])
                nc.sync.dma_start(dbg_outs["d_hb1"][:], hb_rm[:])

            # hb -> hbT, logits
            tpb = ps_t.tile([P, 512], f32, tag="tpw")
            nc.tensor.transpose(tpb[:HB, 384:512], hb_rm[:], ident[:])
            nc.scalar.copy(hbTa[:HB, :], tpb[:HB, 384:512])

            lps = ps_l.tile([P, 2 * K], f32, tag="lps")
            nc.tensor.matmul(lps[:], hbTa[:], w12b[:], start=True, stop=True)
            l12 = wk.tile([P, 2 * K], f32, tag="l12")
            nc.scalar.activation(l12[:], lps[:], AF.Tanh)
            warm = ps_w.tile([P, 64], f32, tag="warm")
            nc.tensor.matmul(warm[:], ones1[:], l12[0:1, 0:64], start=True,
                             stop=True)
            lg12 = wk.tile([P, 2 * K], f32, tag="lg12")
            nc.vector.tensor_tensor(lg12[:], l12[:], g12[:], op=ALU.mult)
            logits = io.tile([P, K], f32, tag="logits")
            nc.vector.tensor_tensor(logits[:], lg12[:, 0:K], lg12[:, K:2 * K],
                                    op=ALU.add)
            nc.sync.dma_start(out_l[t, :, :], logits[:])

            # gumbel softmax expected index
            zz = wk.tile([P, K], f32, tag="zz")
            nc.vector.tensor_tensor(zz[:], logits[:], a2[:], op=ALU.subtract)
            zwarm = ps_w.tile([P, 64], f32, tag="warm")
            nc.tensor.matmul(zwarm[:], ones1[:], zz[0:1, 0:64], start=True,
                             stop=True)
            E = wk.tile([P, K], f32, tag="E")
            den = wk.tile([P, 1], f32, tag="den")
            nc.scalar.activation(E[:], zz[:], AF.Exp, accum_out=den[:])
            warm = ps_w.tile([P, 64], f32, tag="warm")
            nc.tensor.matmul(warm[:], ones1[:], E[0:1, 0:64], start=True,
                             stop=True)
            Escr = wk.tile([P, K], f32, tag="Escr")
            num = wk.tile([P, 1], f32, tag="num")
            nc.vector.scalar_tensor_tensor(Escr[:], E[:], 0.0, idxb[:],
                                           op0=ALU.bypass, op1=ALU.mult,
                                           accum_out=num[:])
            rden = wk.tile([P, 1], f32, tag="rden")
            nc.vector.reciprocal(rden[:], den[:])
            soft = wk.tile([P, 1], f32, tag="soft")
            nc.vector.tensor_tensor(soft[:], num[:], rden[:], op=ALU.mult)
            if dbg and t == 0:
                nc.sync.dma_start(dbg_outs["d_soft0"][:], soft[:])
            # e = round-half-even(soft) == jnp.round (verified on HW)
            eint = wk.tile([P, 1], i32, tag="eint")
            nc.vector.tensor_copy(eint[:], soft[:])
            nc.vector.tensor_copy(dynstage[:, 2:3], eint[:])
            # s = p + e -> ring slot; col1 = p (before tau overwrites col0)
            nc.vector.tensor_tensor(ring[:, t % 16:t % 16 + 1], dynstage[:, 0:1],
                                    dynstage[:, 2:3], op=ALU.add)
            nc.vector.tensor_copy(dynstage[:, 1:2], dynstage[:, 0:1])
            # p(t+1) path
            rot = ((t + 1) % 16) * 16
            sc16 = wk.tile([P, 16], f32, tag="sc16")
            pdot = wk.tile([P, 1], f32, tag="pdot")
            nc.vector.scalar_tensor_tensor(sc16[:], lpcrot[:, rot:rot + 16], 0.0,
                                           ring[:], op0=ALU.bypass, op1=ALU.mult,
                                           accum_out=pdot[:])
            pscr = wk.tile([P, K], f32, tag="pscr")
            nc.vector.tensor_scalar(pscr[:], taub[:], pdot[:], 0.0,
                                    op0=ALU.is_le, op1=ALU.add,
                                    accum_out=dynstage[:, 0:1])
            warm = ps_w.tile([P, 64], f32, tag="warm")
            nc.tensor.matmul(warm[:], ones1[:], pscr[0:1, 0:64], start=True,
                             stop=True)
            if dbg and t == 0:
                nc.sync.dma_start(dbg_outs["d_e0"][:], dynstage[:, 2:3])
                nc.sync.dma_start(dbg_outs["d_s0"][:], ring[:, 0:1])
                nc.sync.dma_start(dbg_outs["d_p0"][:], dynstage[:, 1:2])
                nc.sync.dma_start(dbg_outs["d_p1"][:], dynstage[:, 0:1])
                nc.sync.dma_start(dbg_outs["d_pdot1"][:], pdot[:])

            # open accumulation groups for t+1 (hidden under this step's tail)
            if t + 1 < T_steps:
                garz, ghn, ghbn, gxn, gb = open_groups(dump=(dbg and t == 0))

    return nc, ins, out_l


def _pack_inputs(frames_features, lpc_coeffs, gumbel_u, Wf1, bf1, Wf2, bf2,
                 Wxa, Wha, ba, Wxb, Whb, bb, W1, b1, g1, W2, b2, g2,
                 T_steps=T):
    """Host-side packing -> list of per-core input dicts."""
    import ml_dtypes
    fp = np.float32
    bf = ml_dtypes.bfloat16
    feat = np.ascontiguousarray(frames_features, fp).reshape(R, NF)
    lpc = np.ascontiguousarray(lpc_coeffs, fp).reshape(R, M)
    u = np.ascontiguousarray(gumbel_u, fp)
    gl = np.log(-np.log(u[:T_steps])).astype(fp)

    # lpcrot[:, 16*r + j] = lpc[:, (j - r) % 16]
    lpcrot = np.empty((R, 256), fp)
    for r_ in range(16):
        for j in range(16):
            lpcrot[:, 16 * r_ + j] = lpc[:, (j - r_) % 16]

    wha_s = np.asarray(Wha, fp).copy()
    wha_s[:, 2 * 384:] *= fp(0.5)        # 0.5*gh_n for the tanh-only r-gate
    wha_p = np.concatenate([np.ascontiguousarray(wha_s[c * P:(c + 1) * P, :])
                            for c in range(3)], axis=1)          # [128, 3*1152]
    # wxb chunk c: [rz 32 | n 16]
    wxb_p = np.concatenate(
        [np.concatenate([np.asarray(Wxb, fp)[c * P:(c + 1) * P, 0:32],
                         np.asarray(Wxb, fp)[c * P:(c + 1) * P, 32:48]], axis=1)
         for c in range(3)], axis=1)                             # [128, 144]
    statics = {
        "wf1b": np.concatenate([np.asarray(Wf1, fp), np.asarray(bf1, fp)[None, :]], 0),
        "wf2": np.asarray(Wf2, fp),
        "bf2": np.asarray(bf2, fp)[None, :],
        "wxac": np.ascontiguousarray(np.asarray(Wxa, fp)[:COND, :]),
        "ba": np.asarray(ba, fp)[None, :],
        "wdynh": _wdyn2(np.asarray(Wxa, fp))[0:3].astype(bf),
        "wdynl": _wdyn2(np.asarray(Wxa, fp))[3:6].astype(bf),
        "wha": wha_p,
        "wxb": wxb_p,
        "whbbx": np.concatenate([
            np.concatenate([np.asarray(Whb, fp)[:, 0:32],
                            np.asarray(bb, fp)[None, 0:32]], 0),
            np.concatenate([np.zeros((HB, HB), fp),
                            np.asarray(bb, fp)[None, 32:48]], 0),
            np.concatenate([np.asarray(Whb, fp)[:, 32:48] * fp(0.5),
                            np.zeros((1, HB), fp)], 0)], axis=1),
        "w12b": np.concatenate([
            np.concatenate([np.asarray(W1, fp), np.asarray(W2, fp)], axis=1),
            np.concatenate([np.asarray(b1, fp), np.asarray(b2, fp)])[None, :]],
            0),
        "g12": np.repeat(np.concatenate([np.asarray(g1, fp), np.asarray(g2, fp)])[None, :], P, 0),
        "idxb": np.repeat(np.arange(K, dtype=fp)[None, :], P, 0),
        "ident": np.eye(P, dtype=fp),
        "identr": np.eye(P, dtype=fp),
        "onesrow": np.ones((1, P), fp),
        "onesrowr": np.ones((1, P), fp),
        "zhaT": np.zeros((P, 3 * P), fp),
        "zhbT": np.zeros((HB, P), fp),
        "taub": np.repeat(_tau_table()[None, :], P, 0),
    }
    per_core = []
    for c in range(N_CORES):
        rs = slice(c * P, (c + 1) * P)
        m = dict(statics)
        m["feat"] = np.ascontiguousarray(feat[rs])
        m["lpcrot"] = np.ascontiguousarray(lpcrot[rs])
        m["gl"] = np.ascontiguousarray(gl[:, rs, :])
        per_core.append(m)
    return per_core


def _wdyn2(Wxa):
    """[6, 3H]: bf16 hi/lo split of [w_p, w_s, w_s+w_e] (exact-bf16 dyn matmul)."""
    import ml_dtypes
    fp = np.float32
    wd = Wxa[COND:COND + 3, :].astype(fp).copy()
    wd[2] = (wd[1] + wd[2]).astype(fp)
    hi = wd.astype(ml_dtypes.bfloat16).astype(fp)
    lo = (wd - hi).astype(ml_dtypes.bfloat16).astype(fp)
    return np.concatenate([hi, lo], 0)


def _tau_table():
    """tau[k] = smallest float32 x with mu_law_p(x) >= k+1 (k=0..254);
    tau[255] = +inf sentinel. p(x) = sum_k [x >= tau_k]."""
    fp = np.float32

    def p_of(x):
        x = np.asarray(x, fp)
        xc = np.clip(x, fp(-1.0), fp(1.0)).astype(fp)
        ln_mu1 = np.log(fp(256.0)).astype(fp)
        y = (np.sign(xc) * np.log1p(fp(255.0) * np.abs(xc)) / ln_mu1).astype(fp)
        v = ((y + fp(1.0)) * fp(0.5) * fp(256.0)).astype(fp)
        return np.clip(np.floor(v), 0.0, 255.0)

    def f2i(x):
        b = np.asarray(x, np.float32).view(np.int32)
        return np.where(b < 0, np.int32(-2147483648) - b, b).astype(np.int64)

    def i2f(i):
        i = np.asarray(i, np.int64)
        b = np.where(i < 0, -2147483648 - i, i).astype(np.int32)
        return b.view(np.float32)

    ks = np.arange(1, 256)
    lo = np.full(255, f2i(np.float32(-1.5)), np.int64)
    hi = np.full(255, f2i(np.float32(1.5)), np.int64)
    for _ in range(40):
        mid = (lo + hi) // 2
        ge = p_of(i2f(mid)) >= ks
        hi = np.where(ge, mid, hi)
        lo = np.where(ge, lo, mid)
    tau = i2f(hi).astype(fp)
    out = np.empty(256, fp)
    out[:255] = tau
    out[255] = np.float32(3.0e38)
    return out


_CACHE = {}


def _ensure_devices():
    import jax
    try:
        if len(jax.devices()) >= N_CORES:
            return
    except Exception:
        pass
    jax.config.update("jax_platforms", "axon,cpu")
    import jax.extend.backend as _jeb
    _jeb.clear_backends()
    assert len(jax.devices()) >= N_CORES, (
        f"need {N_CORES} NeuronCores, visible: {jax.devices()}")


def _get_nc(T_steps):
    if T_steps not in _CACHE:
        nc, ins, out_l = _build(T_steps)
        nc.compile()
        _CACHE[T_steps] = nc
    return _CACHE[T_steps]


def kernel(**inputs):
    _ensure_devices()
    nc = _get_nc(T)
    per_core = _pack_inputs(**inputs)
    res = run_bass_kernel_spmd(nc, per_core, list(range(N_CORES)))
    shards = [res.results[c]["logits"] for c in range(N_CORES)]   # each [T,128,K]
    logits_seq = np.concatenate(shards, axis=1)                   # [T, R, K]
    out = logits_seq.transpose(1, 0, 2).reshape(B, F * T, K)
    return np.ascontiguousarray(out, dtype=np.float32)


# revision 30
# speedup vs baseline: 1.1281x; 1.0903x over previous
"""LPCNet sampling kernel for Trainium2 — nn_LPCNet_91061896609827.

kernel(**inputs) takes FULL unsharded inputs (as from reference.setup_inputs())
and returns the FULL [B, F*T, K] float32 logits output.

Strategy: data-parallel over the R = B*F = 1024 row axis, 8 shards of 128 rows
(one per NeuronCore, rows on SBUF partitions), GRU/dense weights replicated.
The T=160 sequential sampling scan runs fully on-device per core.

v2 vs v1 (5.01ms):
  - gumbel g = ln(-ln u) precomputed on HOST; no device prepass, no u DMA.
  - all recurrent matmuls single-pass bf16 (fp32 matmul = 2 PE passes);
    dyn (p/s/e feature) matmul keeps exact bf16 hi/lo split.
  - static gate biases (cond@Wxa+ba, bb) folded into PSUM accumulation via
    identity/ones matmuls -> sigmoid/tanh read PSUM directly, the wide
    vector adds disappear from the serial chain.
  - garz accumulation reordered: the big wha matmuls for step t+1 are
    emitted at the end of step t (hidden under GRU-B/sampling); only the
    tiny dyn matmuls sit on the critical path.
  - round(soft) via direct f32->i32 cast (hardware round-half-even ==
    jnp.round; verified by probe).
  - sigmoid LUT (one op) instead of tanh(0.5x) rescaling (three ops);
    r/z sigmoids split so r is ready earlier.
  - ha transpose copies spread across scalar/vector/gpsimd engines.

Self-contained: hardcodes shapes; reads nothing from /root/problem.
"""
import numpy as np
from contextlib import ExitStack

import concourse.bass as bass
import concourse.tile as tile
import concourse.mybir as mybir
from concourse import bacc
from concourse.bass_utils import run_bass_kernel_spmd

B, F, M, NF = 32, 32, 16, 20
T, K = 160, 256
R = B * F
COND, HA, HB = 128, 384, 16
N_CORES = 8
P = 128  # rows per core == SBUF partitions

f32 = mybir.dt.float32
bf16 = mybir.dt.bfloat16
i32 = mybir.dt.int32
f32r = mybir.dt.float32r
AF = mybir.ActivationFunctionType
ALU = mybir.AluOpType

G3 = 3 * HA  # 1152


def _build(T_steps: int, dbg: bool = False):
    nc = bacc.Bacc("TRN2", target_bir_lowering=False, debug=False,
                   num_devices=N_CORES)

    def din(name, shape, dt=f32):
        return nc.dram_tensor(name, list(shape), dt, kind="ExternalInput").ap()

    dbg_outs = {}
    if dbg:
        for nm, sh in [("d_soft0", [P, 1]), ("d_e0", [P, 1]), ("d_s0", [P, 1]),
                       ("d_p0", [P, 1]), ("d_p1", [P, 1]), ("d_pdot1", [P, 1]),
                       ("d_ha0", [P, HA]), ("d_hb0", [P, HB]),
                       ("d_garz1", [P, 768]), ("d_ghn1", [P, HA]),
                       ("d_gxn1", [P, HA]), ("d_r1", [P, HA]),
                       ("d_an1", [P, HA]), ("d_ha1", [P, HA]),
                       ("d_gb1", [P, 64]), ("d_hb1", [P, HB]),
                       ("d_dynT1", [3, P]), ("d_hbTa1", [HB + 1, P]),
                       ("d_gb1o", [P, 64])]:
            dbg_outs[nm] = nc.dram_tensor(nm, sh, f32, kind="ExternalOutput").ap()

    ins = {
        # per-core
        "feat": din("feat", [P, NF]),
        "lpcrot": din("lpcrot", [P, 16 * 16]),
        "gw": din("gw", [T_steps, P, K]),   # -1/ln(u), host-computed
        "gwi": din("gwi", [T_steps, P, K]),  # idx * -1/ln(u)
        # replicated statics (host-packed)
        "wf1b": din("wf1b", [NF + 1, COND]),
        "wf2": din("wf2", [COND, COND]),
        "bf2": din("bf2", [1, COND]),
        "wxac": din("wxac", [COND, G3]),
        "ba": din("ba", [1, G3]),
        "wdynh": din("wdynh", [3, G3], bf16),
        "wdynl": din("wdynl", [3, G3], bf16),
        "wha": din("wha", [P, 3 * G3], f32r),      # 3 K-chunks on free axis
        "wxb": din("wxb", [P, 3 * 48], f32r),      # chunk c: [rz 32 | n 16]
        "whbbx": din("whbbx", [HB + 1, 64], f32r),  # [[Whb_rz;bb_rz] | [0;bb_n] | [Whb_n;0]]
        "w12b": din("w12b", [HB + 1, 2 * K], f32r),  # [W1|W2 ; b1|b2]
        "g12": din("g12", [P, 2 * K]),             # [g1|g2] row broadcast
        "idxb": din("idxb", [P, K]),               # iota row broadcast
        "ident": din("ident", [P, P]),
        "identr": din("identr", [P, P], f32r),
        "onesrow": din("onesrow", [1, P]),
        "onesrowr": din("onesrowr", [1, P], f32r),
        "zhaT": din("zhaT", [P, 3 * P], f32r),
        "zhbT": din("zhbT", [HB, P], f32r),
        "taub": din("taub", [P, K]),
    }
    out_l = nc.dram_tensor("logits", [T_steps, P, K], f32,
                           kind="ExternalOutput").ap()

    with tile.TileContext(nc) as tc, ExitStack() as ctx:
        st = ctx.enter_context(tc.tile_pool(name="static", bufs=1))
        wk = ctx.enter_context(tc.tile_pool(name="work", bufs=2))
        io = ctx.enter_context(tc.tile_pool(name="io", bufs=3))
        ps_rz = ctx.enter_context(tc.tile_pool(name="ps_rz", bufs=1, space="PSUM"))
        ps_n = ctx.enter_context(tc.tile_pool(name="ps_n", bufs=1, space="PSUM"))
        ps_x = ctx.enter_context(tc.tile_pool(name="ps_x", bufs=1, space="PSUM"))
        ps_b = ctx.enter_context(tc.tile_pool(name="ps_b", bufs=1, space="PSUM"))
        ps_l = ctx.enter_context(tc.tile_pool(name="ps_l", bufs=1, space="PSUM"))
        ps_t = ctx.enter_context(tc.tile_pool(name="ps_t", bufs=1, space="PSUM"))

        # ---- load statics ----
        def load(name, shape, dt=f32, tag=None):
            t_ = st.tile(list(shape), dt, tag=tag or name)
            nc.sync.dma_start(t_[:], ins[name][:])
            return t_

        feat = load("feat", [P, NF])
        lpcrot = load("lpcrot", [P, 256])
        wf1b = load("wf1b", [NF + 1, COND])
        wf2 = load("wf2", [COND, COND])
        bf2 = load("bf2", [1, COND])
        wxac = load("wxac", [COND, G3])
        ba = load("ba", [1, G3])
        wdynh = load("wdynh", [3, G3], bf16)
        wdynl = load("wdynl", [3, G3], bf16)
        wha = load("wha", [P, 3 * G3], f32r)
        wxb = load("wxb", [P, 3 * 48], f32r)
        whbbx = load("whbbx", [HB + 1, 64], f32r)
        w12b = load("w12b", [HB + 1, 2 * K], f32r)
        g12 = load("g12", [P, 2 * K])
        idxb = load("idxb", [P, K])
        ident = load("ident", [P, P])
        identr = load("identr", [P, P], f32r)
        taub = load("taub", [P, K])

        # ---- persistent state ----
        haTw = st.tile([P, 3 * P], f32r, tag="haTw")
        haT = [haTw[:, c * P:(c + 1) * P] for c in range(3)]
        hbTa = st.tile([HB + 1, P], f32r, tag="hbTa")
        ring = st.tile([P, 16], f32, tag="ring")
        dynstage = st.tile([P, 3], f32, tag="dynstage")
        gxs = st.tile([P, G3], f32, tag="gxs")
        gxsr = st.tile([P, G3], f32r, tag="gxsr")

        nc.sync.dma_start(haTw[:], ins["zhaT"][:])
        nc.sync.dma_start(hbTa[:HB, :], ins["zhbT"][:])
        nc.sync.dma_start(hbTa[HB:, :], ins["onesrowr"][:])
        nc.vector.memset(ring[:], 0.0)
        nc.vector.memset(dynstage[:], 0.0)

        # ---- conditioning network (one-time) ----
        ones1 = st.tile([1, P], f32, tag="ones1")
        nc.vector.memset(ones1[:], 1.0)
        tp = ps_t.tile([P, 512], f32, tag="tpw")
        nc.tensor.transpose(tp[:NF, 0:P], feat[:], ident[:])
        featTa = st.tile([NF + 1, P], f32, tag="featTa")
        nc.scalar.copy(featTa[:NF, :], tp[:NF, 0:P])
        nc.sync.dma_start(featTa[NF:, :], ins["onesrow"][:])

        h1ps = ps_l.tile([P, 2 * K], f32, tag="lps")
        nc.tensor.matmul(h1ps[:, :COND], featTa[:], wf1b[:], start=True, stop=True)
        h1 = wk.tile([P, COND], f32, tag="h1")
        nc.scalar.activation(h1[:], h1ps[:, :COND], AF.Tanh)

        tp = ps_t.tile([P, 512], f32, tag="tpw")
        nc.tensor.transpose(tp[:, 0:P], h1[:], ident[:])
        h1T = wk.tile([P, P], f32, tag="h1T")
        nc.scalar.copy(h1T[:], tp[:, 0:P])

        cps = ps_l.tile([P, 2 * K], f32, tag="lps")
        nc.tensor.matmul(cps[:, :COND], h1T[:], wf2[:], start=True, stop=False)
        nc.tensor.matmul(cps[:, :COND], ones1[:], bf2[:], start=False, stop=True)
        cond = wk.tile([P, COND], f32, tag="h1")
        nc.scalar.activation(cond[:], cps[:, :COND], AF.Tanh)

        tp = ps_t.tile([P, 512], f32, tag="tpw")
        nc.tensor.transpose(tp[:, 0:P], cond[:], ident[:])
        condT = wk.tile([P, P], f32, tag="h1T")
        nc.scalar.copy(condT[:], tp[:, 0:P])

        # gxs = cond @ Wxa[:COND] + ba  -> [P, 1152], then cast to bf16
        for sl in ((0, 512), (512, 1024), (1024, G3)):
            gsps = ps_l.tile([P, 2 * K], f32, tag="lps")
            nc.tensor.matmul(gsps[:, :sl[1] - sl[0]], condT[:], wxac[:, sl[0]:sl[1]],
                             start=True, stop=False)
            nc.tensor.matmul(gsps[:, :sl[1] - sl[0]], ones1[:], ba[:, sl[0]:sl[1]],
                             start=False, stop=True)
            nc.vector.tensor_copy(gxs[:, sl[0]:sl[1]], gsps[:, :sl[1] - sl[0]])
            nc.vector.tensor_copy(gxsr[:, sl[0]:sl[1]], gsps[:, :sl[1] - sl[0]])

        # ---- prologue: p(0) path + open accumulation groups for t=0 ----
        pdot = wk.tile([P, 1], f32, tag="pdot")
        sc16 = wk.tile([P, 16], f32, tag="sc16")
        nc.vector.scalar_tensor_tensor(sc16[:], lpcrot[:, 0:16], 0.0, ring[:],
                                       op0=ALU.bypass, op1=ALU.mult,
                                       accum_out=pdot[:])
        pscr = wk.tile([P, K], f32, tag="pscr")
        nc.vector.tensor_scalar(pscr[:], taub[:], pdot[:], 0.0,
                                op0=ALU.is_le, op1=ALU.add,
                                accum_out=dynstage[:, 0:1])

        def open_groups(dump=False):
            """Emit the t+1 accumulations that depend only on haT/hbTa/statics."""
            garz = ps_rz.tile([P, 768], f32, tag="garz")
            psn = ps_n.tile([P, HA + HB], f32, tag="ghn")
            ghn = psn[:, 0:HA]
            ghbn = psn[:, HA:HA + HB]
            gxn = ps_x.tile([P, HA], f32, tag="gxn")
            gb = ps_b.tile([P, 48], f32, tag="gb")
            # rz: gxs + sum_c haT_c @ Wha_c[rz]   (dyn closes later)
            # (single-matmul output is capped at one PSUM bank: 512 fp32)
            nc.tensor.matmul(garz[:, 0:512], identr[:], gxsr[:, 0:512],
                             start=True, stop=False)
            nc.tensor.matmul(garz[:, 512:768], identr[:], gxsr[:, 512:768],
                             start=True, stop=False)
            for c in range(3):
                w0 = c * G3
                nc.tensor.matmul(garz[:, 0:512], haT[c], wha[:, w0:w0 + 512],
                                 start=False, stop=False)
                nc.tensor.matmul(garz[:, 512:768], haT[c],
                                 wha[:, w0 + 512:w0 + 768],
                                 start=False, stop=False)
            # n (h-part): sum_c haT_c @ Wha_c[n]  (closed here)
            for c in range(3):
                w0 = c * G3
                nc.tensor.matmul(ghn[:], haT[c], wha[:, w0 + 768:w0 + G3],
                                 start=(c == 0), stop=(c == 2))
            # n (x-part): gxs_n  (dyn closes later)
            nc.tensor.matmul(gxn[:], identr[:], gxsr[:, 768:G3],
                             start=True, stop=False)
            # GRU-B: biases folded into the hbTa matmul (ones row of hbTa).
            # One accumulation group per PSUM bank: start=True clears the
            # whole bank's has_written bits, so ghb_n lives in the ps_n bank
            # (whose groups are emitted before it) and gb holds one group.
            nc.tensor.matmul(gb[:, 0:48], hbTa[:], whbbx[:, 0:48],
                             start=True, stop=False)
            nc.tensor.matmul(ghbn[:], hbTa[:], whbbx[:, 48:64],
                             start=True, stop=True)
            if dump:
                nc.sync.dma_start(dbg_outs["d_hbTa1"][:], hbTa[:])
                gbo_c = wk.tile([P, 64], f32, tag="dbg_gbo")
                nc.vector.tensor_copy(gbo_c[:, 0:48], gb[:])
                nc.vector.tensor_copy(gbo_c[:, 48:64], ghbn[:])
                nc.sync.dma_start(dbg_outs["d_gb1o"][:], gbo_c[:])
            return garz, ghn, ghbn, gxn, gb

        garz, ghn, ghbn, gxn, gb = open_groups()

        ha_rm = wk.tile([P, HA], f32, tag="ha_rm")
        nc.vector.memset(ha_rm[:], 0.0)
        hb_rm = wk.tile([P, HB], f32, tag="hb_rm")
        nc.vector.memset(hb_rm[:], 0.0)

        # ---- time loop ----
        for t in range(T_steps):
            # gumbel prefetch
            a2 = io.tile([P, K], f32, tag="a2")
            nc.sync.dma_start(a2[:], ins["gw"][t, :, :])
            a3 = io.tile([P, K], f32, tag="a3")
            nc.sync.dma_start(a3[:], ins["gwi"][t, :, :])

            # dyn features -> transposed [3, P]
            tpd = ps_t.tile([P, 512], f32, tag="tpw")
            nc.tensor.transpose(tpd[:3, 384:512], dynstage[:], ident[:])
            dynT = wk.tile([3, P], bf16, tag="dynT")
            nc.vector.tensor_copy(dynT[:], tpd[:3, 384:512])
            if dbg and t == 1:
                dynTf = wk.tile([3, P], f32, tag="dynTf")
                nc.vector.tensor_copy(dynTf[:], dynT[:])
                nc.sync.dma_start(dbg_outs["d_dynT1"][:], dynTf[:])

            # dyn matmuls close the rz / gxn groups (exact bf16 hi/lo)
            nc.tensor.matmul(garz[:, 0:512], dynT[:], wdynh[:, 0:512],
                             start=False, stop=False)
            nc.tensor.matmul(garz[:, 0:512], dynT[:], wdynl[:, 0:512],
                             start=False, stop=True)
            nc.tensor.matmul(garz[:, 512:768], dynT[:], wdynh[:, 512:768],
                             start=False, stop=False)
            nc.tensor.matmul(garz[:, 512:768], dynT[:], wdynl[:, 512:768],
                             start=False, stop=True)
            nc.tensor.matmul(gxn[:], dynT[:], wdynh[:, 768:G3],
                             start=False, stop=False)
            nc.tensor.matmul(gxn[:], dynT[:], wdynl[:, 768:G3],
                             start=False, stop=True)

            # GRU-A gates, tanh-only (sigmoid via th=tanh(x/2):
            # r*ghn == (th_r+1)*(0.5*ghn), 0.5 folded into Wha_n host-side;
            # z-blend: ha2 = an + z*(ha-an) == 0.5*((th_z+1)*(ha-an)) + an)
            thr = wk.tile([P, HA], f32, tag="thr")
            nc.scalar.activation(thr[:], garz[:, 0:HA], AF.Tanh, scale=0.5)
            thz = wk.tile([P, HA], f32, tag="thz")
            nc.scalar.activation(thz[:], garz[:, HA:768], AF.Tanh, scale=0.5)
            t1 = wk.tile([P, HA], f32, tag="t1")
            nc.vector.scalar_tensor_tensor(t1[:], thr[:], 1.0, ghn[:],
                                           op0=ALU.add, op1=ALU.mult)
            t3 = wk.tile([P, HA], f32, tag="t3")
            nc.vector.tensor_tensor(t3[:], t1[:], gxn[:], op=ALU.add)
            an = wk.tile([P, HA], f32, tag="an")
            nc.scalar.activation(an[:], t3[:], AF.Tanh)
            d = wk.tile([P, HA], f32, tag="d")
            nc.vector.tensor_tensor(d[:], ha_rm[:], an[:], op=ALU.subtract)
            zd = wk.tile([P, HA], f32, tag="zd")
            nc.vector.scalar_tensor_tensor(zd[:], thz[:], 1.0, d[:],
                                           op0=ALU.add, op1=ALU.mult)
            ha_rm = wk.tile([P, HA], f32, tag="ha_rm")
            nc.vector.scalar_tensor_tensor(ha_rm[:], zd[:], 0.5, an[:],
                                           op0=ALU.mult, op1=ALU.add)
            if dbg and t == 0:
                nc.sync.dma_start(dbg_outs["d_ha0"][:], ha_rm[:])
            if dbg and t == 1:
                garz_c = wk.tile([P, 768], f32, tag="dbg_garz")
                nc.vector.tensor_copy(garz_c[:], garz[:])
                nc.sync.dma_start(dbg_outs["d_garz1"][:], garz_c[:])
                ghn_c = wk.tile([P, HA], f32, tag="dbg_ghn")
                nc.vector.tensor_copy(ghn_c[:], ghn[:])
                nc.sync.dma_start(dbg_outs["d_ghn1"][:], ghn_c[:])
                gxn_c = wk.tile([P, HA], f32, tag="dbg_gxn")
                nc.vector.tensor_copy(gxn_c[:], gxn[:])
                nc.sync.dma_start(dbg_outs["d_gxn1"][:], gxn_c[:])
                nc.sync.dma_start(dbg_outs["d_r1"][:], r[:])
                nc.sync.dma_start(dbg_outs["d_an1"][:], an[:])
                nc.sync.dma_start(dbg_outs["d_ha1"][:], ha_rm[:])

            # ha2 -> haT: transposes back-to-back on PE, copies on
            # scalar/vector, then the gxb matmuls
            tpc = ps_t.tile([P, 512], f32, tag="tpw")
            for c in range(3):
                nc.tensor.transpose(tpc[:, c * P:(c + 1) * P],
                                    ha_rm[:, c * P:(c + 1) * P], ident[:])
            nc.scalar.copy(haTw[:, 0:P], tpc[:, 0:P])
            nc.vector.tensor_copy(haTw[:, P:2 * P], tpc[:, P:2 * P])
            nc.scalar.copy(haTw[:, 2 * P:3 * P], tpc[:, 2 * P:3 * P])
            for c in range(3):
                nc.tensor.matmul(gb[:, 0:48], haT[c], wxb[:, c * 48:(c + 1) * 48],
                                 start=False, stop=(c == 2))

            # GRU-B gates (same tanh-only scheme; 0.5 folded into whbbx_n)
            thb = wk.tile([P, 2 * HB], f32, tag="thb")
            nc.scalar.activation(thb[:], gb[:, 0:32], AF.Tanh, scale=0.5)
            t1b = wk.tile([P, HB], f32, tag="t1b")
            nc.vector.scalar_tensor_tensor(t1b[:], thb[:, 0:HB], 1.0, ghbn[:],
                                           op0=ALU.add, op1=ALU.mult)
            t2b = wk.tile([P, HB], f32, tag="t2b")
            nc.vector.tensor_tensor(t2b[:], t1b[:], gb[:, 32:48], op=ALU.add)
            nb = wk.tile([P, HB], f32, tag="nb")
            nc.scalar.activation(nb[:], t2b[:], AF.Tanh)
            db = wk.tile([P, HB], f32, tag="db")
            nc.vector.tensor_tensor(db[:], hb_rm[:], nb[:], op=ALU.subtract)
            zdb = wk.tile([P, HB], f32, tag="zdb")
            nc.vector.scalar_tensor_tensor(zdb[:], thb[:, HB:2 * HB], 1.0, db[:],
                                           op0=ALU.add, op1=ALU.mult)
            hb_rm = wk.tile([P, HB], f32, tag="hb_rm")
            nc.vector.scalar_tensor_tensor(hb_rm[:], zdb[:], 0.5, nb[:],
                                           op0=ALU.mult, op1=ALU.add)
            if dbg and t == 0:
                nc.sync.dma_start(dbg_outs["d_hb0"][:], hb_rm[:])
            if dbg and t == 1:
                gb_c = wk.tile([P, 64], f32, tag="dbg_gb")
                nc.vector.tensor_copy(gb_c[:, 0:48], gb[:])
                nc.vector.tensor_copy(gb_c[:, 48:64], ghbn[:])
                nc.sync.dma_start(dbg_outs["d_gb1"][:], gb_c[:])
                nc.sync.dma_start(dbg_outs["d_hb1"][:], hb_rm[:])

            # hb -> hbT, logits
            tpb = ps_t.tile([P, 512], f32, tag="tpw")
            nc.tensor.transpose(tpb[:HB, 384:512], hb_rm[:], ident[:])
            nc.scalar.copy(hbTa[:HB, :], tpb[:HB, 384:512])

            lps = ps_l.tile([P, 2 * K], f32, tag="lps")
            nc.tensor.matmul(lps[:], hbTa[:], w12b[:], start=True, stop=True)
            l12 = wk.tile([P, 2 * K], f32, tag="l12")
            nc.scalar.activation(l12[:], lps[:], AF.Tanh)
            lg12 = wk.tile([P, 2 * K], f32, tag="lg12")
            nc.vector.tensor_tensor(lg12[:], l12[:], g12[:], op=ALU.mult)
            logits = io.tile([P, K], f32, tag="logits")
            nc.vector.tensor_tensor(logits[:], lg12[:, 0:K], lg12[:, K:2 * K],
                                    op=ALU.add)
            nc.sync.dma_start(out_l[t, :, :], logits[:])

            # gumbel softmax expected index:
            # probs ~ exp(logits - ln(-ln u)) = exp(logits) * w,
            # w = -1/ln(u) and w*idx precomputed on host (a2, a3)
            E = wk.tile([P, K], f32, tag="E")
            nc.scalar.activation(E[:], logits[:], AF.Exp)
            Ew = wk.tile([P, K], f32, tag="Ew")
            den = wk.tile([P, 1], f32, tag="den")
            nc.vector.scalar_tensor_tensor(Ew[:], E[:], 0.0, a2[:],
                                           op0=ALU.bypass, op1=ALU.mult,
                                           accum_out=den[:])
            Escr = wk.tile([P, K], f32, tag="Escr")
            num = wk.tile([P, 1], f32, tag="num")
            nc.vector.scalar_tensor_tensor(Escr[:], E[:], 0.0, a3[:],
                                           op0=ALU.bypass, op1=ALU.mult,
                                           accum_out=num[:])
            rden = wk.tile([P, 1], f32, tag="rden")
            nc.vector.reciprocal(rden[:], den[:])
            # e = round-half-even(num/den) == jnp.round, via i32-output mult
            eint = wk.tile([P, 1], i32, tag="eint")
            nc.vector.tensor_tensor(eint[:], num[:], rden[:], op=ALU.mult)
            nc.vector.tensor_copy(dynstage[:, 2:3], eint[:])
            # s = p + e -> ring slot; col1 = p (before tau overwrites col0)
            nc.vector.tensor_tensor(ring[:, t % 16:t % 16 + 1], dynstage[:, 0:1],
                                    dynstage[:, 2:3], op=ALU.add)
            nc.vector.tensor_copy(dynstage[:, 1:2], dynstage[:, 0:1])
            # p(t+1) path
            rot = ((t + 1) % 16) * 16
            sc16 = wk.tile([P, 16], f32, tag="sc16")
            pdot = wk.tile([P, 1], f32, tag="pdot")
            nc.vector.scalar_tensor_tensor(sc16[:], lpcrot[:, rot:rot + 16], 0.0,
                                           ring[:], op0=ALU.bypass, op1=ALU.mult,
                                           accum_out=pdot[:])
            pscr = wk.tile([P, K], f32, tag="pscr")
            nc.vector.tensor_scalar(pscr[:], taub[:], pdot[:], 0.0,
                                    op0=ALU.is_le, op1=ALU.add,
                                    accum_out=dynstage[:, 0:1])

            if dbg and t == 0:
                nc.sync.dma_start(dbg_outs["d_e0"][:], dynstage[:, 2:3])
                nc.sync.dma_start(dbg_outs["d_s0"][:], ring[:, 0:1])
                nc.sync.dma_start(dbg_outs["d_p0"][:], dynstage[:, 1:2])
                nc.sync.dma_start(dbg_outs["d_p1"][:], dynstage[:, 0:1])
                nc.sync.dma_start(dbg_outs["d_pdot1"][:], pdot[:])

            # open accumulation groups for t+1 (hidden under this step's tail)
            if t + 1 < T_steps:
                garz, ghn, ghbn, gxn, gb = open_groups(dump=(dbg and t == 0))

    return nc, ins, out_l


def _pack_inputs(frames_features, lpc_coeffs, gumbel_u, Wf1, bf1, Wf2, bf2,
                 Wxa, Wha, ba, Wxb, Whb, bb, W1, b1, g1, W2, b2, g2,
                 T_steps=T):
    """Host-side packing -> list of per-core input dicts."""
    import ml_dtypes
    fp = np.float32
    bf = ml_dtypes.bfloat16
    feat = np.ascontiguousarray(frames_features, fp).reshape(R, NF)
    lpc = np.ascontiguousarray(lpc_coeffs, fp).reshape(R, M)
    u = np.ascontiguousarray(gumbel_u, fp)
    gw = (-1.0 / np.log(u[:T_steps])).astype(fp)
    gwi = (gw * np.arange(K, dtype=fp)[None, None, :]).astype(fp)

    # lpcrot[:, 16*r + j] = lpc[:, (j - r) % 16]
    lpcrot = np.empty((R, 256), fp)
    for r_ in range(16):
        for j in range(16):
            lpcrot[:, 16 * r_ + j] = lpc[:, (j - r_) % 16]

    wha_s = np.asarray(Wha, fp).copy()
    wha_s[:, 2 * 384:] *= fp(0.5)        # 0.5*gh_n for the tanh-only r-gate
    wha_p = np.concatenate([np.ascontiguousarray(wha_s[c * P:(c + 1) * P, :])
                            for c in range(3)], axis=1)          # [128, 3*1152]
    # wxb chunk c: [rz 32 | n 16]
    wxb_p = np.concatenate(
        [np.concatenate([np.asarray(Wxb, fp)[c * P:(c + 1) * P, 0:32],
                         np.asarray(Wxb, fp)[c * P:(c + 1) * P, 32:48]], axis=1)
         for c in range(3)], axis=1)                             # [128, 144]
    statics = {
        "wf1b": np.concatenate([np.asarray(Wf1, fp), np.asarray(bf1, fp)[None, :]], 0),
        "wf2": np.asarray(Wf2, fp),
        "bf2": np.asarray(bf2, fp)[None, :],
        "wxac": np.ascontiguousarray(np.asarray(Wxa, fp)[:COND, :]),
        "ba": np.asarray(ba, fp)[None, :],
        "wdynh": _wdyn2(np.asarray(Wxa, fp))[0:3].astype(bf),
        "wdynl": _wdyn2(np.asarray(Wxa, fp))[3:6].astype(bf),
        "wha": wha_p,
        "wxb": wxb_p,
        "whbbx": np.concatenate([
            np.concatenate([np.asarray(Whb, fp)[:, 0:32],
                            np.asarray(bb, fp)[None, 0:32]], 0),
            np.concatenate([np.zeros((HB, HB), fp),
                            np.asarray(bb, fp)[None, 32:48]], 0),
            np.concatenate([np.asarray(Whb, fp)[:, 32:48] * fp(0.5),
                            np.zeros((1, HB), fp)], 0)], axis=1),
        "w12b": np.concatenate([
            np.concatenate([np.asarray(W1, fp), np.asarray(W2, fp)], axis=1),
            np.concatenate([np.asarray(b1, fp), np.asarray(b2, fp)])[None, :]],
            0),
        "g12": np.repeat(np.concatenate([np.asarray(g1, fp), np.asarray(g2, fp)])[None, :], P, 0),
        "idxb": np.repeat(np.arange(K, dtype=fp)[None, :], P, 0),
        "ident": np.eye(P, dtype=fp),
        "identr": np.eye(P, dtype=fp),
        "onesrow": np.ones((1, P), fp),
        "onesrowr": np.ones((1, P), fp),
        "zhaT": np.zeros((P, 3 * P), fp),
        "zhbT": np.zeros((HB, P), fp),
        "taub": np.repeat(_tau_table()[None, :], P, 0),
    }
    per_core = []
    for c in range(N_CORES):
        rs = slice(c * P, (c + 1) * P)
        m = dict(statics)
        m["feat"] = np.ascontiguousarray(feat[rs])
        m["lpcrot"] = np.ascontiguousarray(lpcrot[rs])
        m["gw"] = np.ascontiguousarray(gw[:, rs, :])
        m["gwi"] = np.ascontiguousarray(gwi[:, rs, :])
        per_core.append(m)
    return per_core


def _wdyn2(Wxa):
    """[6, 3H]: bf16 hi/lo split of [w_p, w_s, w_s+w_e] (exact-bf16 dyn matmul)."""
    import ml_dtypes
    fp = np.float32
    wd = Wxa[COND:COND + 3, :].astype(fp).copy()
    wd[2] = (wd[1] + wd[2]).astype(fp)
    hi = wd.astype(ml_dtypes.bfloat16).astype(fp)
    lo = (wd - hi).astype(ml_dtypes.bfloat16).astype(fp)
    return np.concatenate([hi, lo], 0)


def _tau_table():
    """tau[k] = smallest float32 x with mu_law_p(x) >= k+1 (k=0..254);
    tau[255] = +inf sentinel. p(x) = sum_k [x >= tau_k]."""
    fp = np.float32

    def p_of(x):
        x = np.asarray(x, fp)
        xc = np.clip(x, fp(-1.0), fp(1.0)).astype(fp)
        ln_mu1 = np.log(fp(256.0)).astype(fp)
        y = (np.sign(xc) * np.log1p(fp(255.0) * np.abs(xc)) / ln_mu1).astype(fp)
        v = ((y + fp(1.0)) * fp(0.5) * fp(256.0)).astype(fp)
        return np.clip(np.floor(v), 0.0, 255.0)

    def f2i(x):
        b = np.asarray(x, np.float32).view(np.int32)
        return np.where(b < 0, np.int32(-2147483648) - b, b).astype(np.int64)

    def i2f(i):
        i = np.asarray(i, np.int64)
        b = np.where(i < 0, -2147483648 - i, i).astype(np.int32)
        return b.view(np.float32)

    ks = np.arange(1, 256)
    lo = np.full(255, f2i(np.float32(-1.5)), np.int64)
    hi = np.full(255, f2i(np.float32(1.5)), np.int64)
    for _ in range(40):
        mid = (lo + hi) // 2
        ge = p_of(i2f(mid)) >= ks
        hi = np.where(ge, mid, hi)
        lo = np.where(ge, lo, mid)
    tau = i2f(hi).astype(fp)
    out = np.empty(256, fp)
    out[:255] = tau
    out[255] = np.float32(3.0e38)
    return out


_CACHE = {}


def _ensure_devices():
    import jax
    try:
        if len(jax.devices()) >= N_CORES:
            return
    except Exception:
        pass
    jax.config.update("jax_platforms", "axon,cpu")
    import jax.extend.backend as _jeb
    _jeb.clear_backends()
    assert len(jax.devices()) >= N_CORES, (
        f"need {N_CORES} NeuronCores, visible: {jax.devices()}")


def _get_nc(T_steps):
    if T_steps not in _CACHE:
        nc, ins, out_l = _build(T_steps)
        nc.compile()
        _CACHE[T_steps] = nc
    return _CACHE[T_steps]


def kernel(**inputs):
    _ensure_devices()
    nc = _get_nc(T)
    per_core = _pack_inputs(**inputs)
    res = run_bass_kernel_spmd(nc, per_core, list(range(N_CORES)))
    shards = [res.results[c]["logits"] for c in range(N_CORES)]   # each [T,128,K]
    logits_seq = np.concatenate(shards, axis=1)                   # [T, R, K]
    out = logits_seq.transpose(1, 0, 2).reshape(B, F * T, K)
    return np.ascontiguousarray(out, dtype=np.float32)


# revision 31
# speedup vs baseline: 1.1596x; 1.0280x over previous
"""LPCNet sampling kernel for Trainium2 — nn_LPCNet_91061896609827.

kernel(**inputs) takes FULL unsharded inputs (as from reference.setup_inputs())
and returns the FULL [B, F*T, K] float32 logits output.

Strategy: data-parallel over the R = B*F = 1024 row axis, 8 shards of 128 rows
(one per NeuronCore, rows on SBUF partitions), GRU/dense weights replicated.
The T=160 sequential sampling scan runs fully on-device per core.

v2 vs v1 (5.01ms):
  - gumbel g = ln(-ln u) precomputed on HOST; no device prepass, no u DMA.
  - all recurrent matmuls single-pass bf16 (fp32 matmul = 2 PE passes);
    dyn (p/s/e feature) matmul keeps exact bf16 hi/lo split.
  - static gate biases (cond@Wxa+ba, bb) folded into PSUM accumulation via
    identity/ones matmuls -> sigmoid/tanh read PSUM directly, the wide
    vector adds disappear from the serial chain.
  - garz accumulation reordered: the big wha matmuls for step t+1 are
    emitted at the end of step t (hidden under GRU-B/sampling); only the
    tiny dyn matmuls sit on the critical path.
  - round(soft) via direct f32->i32 cast (hardware round-half-even ==
    jnp.round; verified by probe).
  - sigmoid LUT (one op) instead of tanh(0.5x) rescaling (three ops);
    r/z sigmoids split so r is ready earlier.
  - ha transpose copies spread across scalar/vector/gpsimd engines.

Self-contained: hardcodes shapes; reads nothing from /root/problem.
"""
import numpy as np
from contextlib import ExitStack

import concourse.bass as bass
import concourse.tile as tile
import concourse.mybir as mybir
from concourse import bacc
from concourse.bass_utils import run_bass_kernel_spmd

B, F, M, NF = 32, 32, 16, 20
T, K = 160, 256
R = B * F
COND, HA, HB = 128, 384, 16
N_CORES = 8
P = 128  # rows per core == SBUF partitions

f32 = mybir.dt.float32
bf16 = mybir.dt.bfloat16
i32 = mybir.dt.int32
f32r = mybir.dt.float32r
AF = mybir.ActivationFunctionType
ALU = mybir.AluOpType

G3 = 3 * HA  # 1152


def _build(T_steps: int, dbg: bool = False):
    nc = bacc.Bacc("TRN2", target_bir_lowering=False, debug=False,
                   num_devices=N_CORES)

    def din(name, shape, dt=f32):
        return nc.dram_tensor(name, list(shape), dt, kind="ExternalInput").ap()

    dbg_outs = {}
    if dbg:
        for nm, sh in [("d_soft0", [P, 1]), ("d_e0", [P, 1]), ("d_s0", [P, 1]),
                       ("d_p0", [P, 1]), ("d_p1", [P, 1]), ("d_pdot1", [P, 1]),
                       ("d_ha0", [P, HA]), ("d_hb0", [P, HB]),
                       ("d_garz1", [P, 768]), ("d_ghn1", [P, HA]),
                       ("d_gxn1", [P, HA]), ("d_r1", [P, HA]),
                       ("d_an1", [P, HA]), ("d_ha1", [P, HA]),
                       ("d_gb1", [P, 64]), ("d_hb1", [P, HB]),
                       ("d_dynT1", [3, P]), ("d_hbTa1", [HB + 1, P]),
                       ("d_gb1o", [P, 64])]:
            dbg_outs[nm] = nc.dram_tensor(nm, sh, f32, kind="ExternalOutput").ap()

    ins = {
        # per-core
        "feat": din("feat", [P, NF]),
        "lpcrot": din("lpcrot", [P, 16 * 16]),
        "gw": din("gw", [T_steps, P, K]),   # -1/ln(u), host-computed
        "gwi": din("gwi", [T_steps, P, K]),  # idx * -1/ln(u)
        # replicated statics (host-packed)
        "wf1b": din("wf1b", [NF + 1, COND]),
        "wf2": din("wf2", [COND, COND]),
        "bf2": din("bf2", [1, COND]),
        "wxac": din("wxac", [COND, G3]),
        "ba": din("ba", [1, G3]),
        "wdynr": din("wdynr", [3, G3], f32r),
        "wha": din("wha", [P, 3 * G3], f32r),      # 3 K-chunks on free axis
        "wxb": din("wxb", [P, 3 * 48], f32r),      # chunk c: [rz 32 | n 16]
        "whbbx": din("whbbx", [HB + 1, 64], f32r),  # [[Whb_rz;bb_rz] | [0;bb_n] | [Whb_n;0]]
        "w12b": din("w12b", [HB + 1, 2 * K], f32r),  # [W1|W2 ; b1|b2]
        "g12": din("g12", [P, 2 * K]),             # [g1|g2] row broadcast
        "idxb": din("idxb", [P, K]),               # iota row broadcast
        "ident": din("ident", [P, P]),
        "identr": din("identr", [P, P], f32r),
        "onesrow": din("onesrow", [1, P]),
        "onesrowr": din("onesrowr", [1, P], f32r),
        "zhaT": din("zhaT", [P, 3 * P], f32r),
        "zhbT": din("zhbT", [HB, P], f32r),
        "taub": din("taub", [P, K]),
    }
    out_l = nc.dram_tensor("logits", [T_steps, P, K], f32,
                           kind="ExternalOutput").ap()

    with tile.TileContext(nc) as tc, ExitStack() as ctx:
        st = ctx.enter_context(tc.tile_pool(name="static", bufs=1))
        wk = ctx.enter_context(tc.tile_pool(name="work", bufs=2))
        io = ctx.enter_context(tc.tile_pool(name="io", bufs=3))
        ps_rz = ctx.enter_context(tc.tile_pool(name="ps_rz", bufs=1, space="PSUM"))
        ps_n = ctx.enter_context(tc.tile_pool(name="ps_n", bufs=1, space="PSUM"))
        ps_x = ctx.enter_context(tc.tile_pool(name="ps_x", bufs=1, space="PSUM"))
        ps_b = ctx.enter_context(tc.tile_pool(name="ps_b", bufs=1, space="PSUM"))
        ps_l = ctx.enter_context(tc.tile_pool(name="ps_l", bufs=1, space="PSUM"))
        ps_t = ctx.enter_context(tc.tile_pool(name="ps_t", bufs=1, space="PSUM"))

        # ---- load statics ----
        def load(name, shape, dt=f32, tag=None):
            t_ = st.tile(list(shape), dt, tag=tag or name)
            nc.sync.dma_start(t_[:], ins[name][:])
            return t_

        feat = load("feat", [P, NF])
        lpcrot = load("lpcrot", [P, 256])
        wf1b = load("wf1b", [NF + 1, COND])
        wf2 = load("wf2", [COND, COND])
        bf2 = load("bf2", [1, COND])
        wxac = load("wxac", [COND, G3])
        ba = load("ba", [1, G3])
        wdynr = load("wdynr", [3, G3], f32r)
        wha = load("wha", [P, 3 * G3], f32r)
        wxb = load("wxb", [P, 3 * 48], f32r)
        whbbx = load("whbbx", [HB + 1, 64], f32r)
        w12b = load("w12b", [HB + 1, 2 * K], f32r)
        g12 = load("g12", [P, 2 * K])
        idxb = load("idxb", [P, K])
        ident = load("ident", [P, P])
        identr = load("identr", [P, P], f32r)
        taub = load("taub", [P, K])

        # ---- persistent state ----
        haTw = st.tile([P, 3 * P], f32r, tag="haTw")
        haT = [haTw[:, c * P:(c + 1) * P] for c in range(3)]
        hbTa = st.tile([HB + 1, P], f32r, tag="hbTa")
        ring = st.tile([P, 16], f32, tag="ring")
        dynstage = st.tile([P, 3], f32, tag="dynstage")
        gxs = st.tile([P, G3], f32, tag="gxs")
        gxsr = st.tile([P, G3], f32r, tag="gxsr")

        nc.sync.dma_start(haTw[:], ins["zhaT"][:])
        nc.sync.dma_start(hbTa[:HB, :], ins["zhbT"][:])
        nc.sync.dma_start(hbTa[HB:, :], ins["onesrowr"][:])
        nc.vector.memset(ring[:], 0.0)
        nc.vector.memset(dynstage[:], 0.0)

        # ---- conditioning network (one-time) ----
        ones1 = st.tile([1, P], f32, tag="ones1")
        nc.vector.memset(ones1[:], 1.0)
        tp = ps_t.tile([P, 512], f32, tag="tpw")
        nc.tensor.transpose(tp[:NF, 0:P], feat[:], ident[:])
        featTa = st.tile([NF + 1, P], f32, tag="featTa")
        nc.scalar.copy(featTa[:NF, :], tp[:NF, 0:P])
        nc.sync.dma_start(featTa[NF:, :], ins["onesrow"][:])

        h1ps = ps_l.tile([P, 2 * K], f32, tag="lps")
        nc.tensor.matmul(h1ps[:, :COND], featTa[:], wf1b[:], start=True, stop=True)
        h1 = wk.tile([P, COND], f32, tag="h1")
        nc.scalar.activation(h1[:], h1ps[:, :COND], AF.Tanh)

        tp = ps_t.tile([P, 512], f32, tag="tpw")
        nc.tensor.transpose(tp[:, 0:P], h1[:], ident[:])
        h1T = wk.tile([P, P], f32, tag="h1T")
        nc.scalar.copy(h1T[:], tp[:, 0:P])

        cps = ps_l.tile([P, 2 * K], f32, tag="lps")
        nc.tensor.matmul(cps[:, :COND], h1T[:], wf2[:], start=True, stop=False)
        nc.tensor.matmul(cps[:, :COND], ones1[:], bf2[:], start=False, stop=True)
        cond = wk.tile([P, COND], f32, tag="h1")
        nc.scalar.activation(cond[:], cps[:, :COND], AF.Tanh)

        tp = ps_t.tile([P, 512], f32, tag="tpw")
        nc.tensor.transpose(tp[:, 0:P], cond[:], ident[:])
        condT = wk.tile([P, P], f32, tag="h1T")
        nc.scalar.copy(condT[:], tp[:, 0:P])

        # gxs = cond @ Wxa[:COND] + ba  -> [P, 1152], then cast to bf16
        for sl in ((0, 512), (512, 1024), (1024, G3)):
            gsps = ps_l.tile([P, 2 * K], f32, tag="lps")
            nc.tensor.matmul(gsps[:, :sl[1] - sl[0]], condT[:], wxac[:, sl[0]:sl[1]],
                             start=True, stop=False)
            nc.tensor.matmul(gsps[:, :sl[1] - sl[0]], ones1[:], ba[:, sl[0]:sl[1]],
                             start=False, stop=True)
            nc.vector.tensor_copy(gxs[:, sl[0]:sl[1]], gsps[:, :sl[1] - sl[0]])
            nc.vector.tensor_copy(gxsr[:, sl[0]:sl[1]], gsps[:, :sl[1] - sl[0]])

        # ---- prologue: p(0) path + open accumulation groups for t=0 ----
        pdot = wk.tile([P, 1], f32, tag="pdot")
        sc16 = wk.tile([P, 16], f32, tag="sc16")
        nc.vector.scalar_tensor_tensor(sc16[:], lpcrot[:, 0:16], 0.0, ring[:],
                                       op0=ALU.bypass, op1=ALU.mult,
                                       accum_out=pdot[:])
        pscr = wk.tile([P, K], f32, tag="pscr")
        nc.vector.tensor_scalar(pscr[:], taub[:], pdot[:], 0.0,
                                op0=ALU.is_le, op1=ALU.add,
                                accum_out=dynstage[:, 0:1])

        def open_groups(dump=False):
            """Emit the t+1 accumulations that depend only on haT/hbTa/statics."""
            garz = ps_rz.tile([P, 768], f32, tag="garz")
            psn = ps_n.tile([P, HA + HB], f32, tag="ghn")
            ghn = psn[:, 0:HA]
            ghbn = psn[:, HA:HA + HB]
            gxn = ps_x.tile([P, HA], f32, tag="gxn")
            gb = ps_b.tile([P, 48], f32, tag="gb")
            # rz: gxs + sum_c haT_c @ Wha_c[rz]   (dyn closes later)
            # (single-matmul output is capped at one PSUM bank: 512 fp32)
            nc.tensor.matmul(garz[:, 0:512], identr[:], gxsr[:, 0:512],
                             start=True, stop=False)
            nc.tensor.matmul(garz[:, 512:768], identr[:], gxsr[:, 512:768],
                             start=True, stop=False)
            for c in range(3):
                w0 = c * G3
                nc.tensor.matmul(garz[:, 0:512], haT[c], wha[:, w0:w0 + 512],
                                 start=False, stop=False)
                nc.tensor.matmul(garz[:, 512:768], haT[c],
                                 wha[:, w0 + 512:w0 + 768],
                                 start=False, stop=False)
            # n (h-part): sum_c haT_c @ Wha_c[n]  (closed here)
            for c in range(3):
                w0 = c * G3
                nc.tensor.matmul(ghn[:], haT[c], wha[:, w0 + 768:w0 + G3],
                                 start=(c == 0), stop=(c == 2))
            # n (x-part): gxs_n  (dyn closes later)
            nc.tensor.matmul(gxn[:], identr[:], gxsr[:, 768:G3],
                             start=True, stop=False)
            # GRU-B: biases folded into the hbTa matmul (ones row of hbTa).
            # One accumulation group per PSUM bank: start=True clears the
            # whole bank's has_written bits, so ghb_n lives in the ps_n bank
            # (whose groups are emitted before it) and gb holds one group.
            nc.tensor.matmul(gb[:, 0:48], hbTa[:], whbbx[:, 0:48],
                             start=True, stop=False)
            nc.tensor.matmul(ghbn[:], hbTa[:], whbbx[:, 48:64],
                             start=True, stop=True)
            if dump:
                nc.sync.dma_start(dbg_outs["d_hbTa1"][:], hbTa[:])
                gbo_c = wk.tile([P, 64], f32, tag="dbg_gbo")
                nc.vector.tensor_copy(gbo_c[:, 0:48], gb[:])
                nc.vector.tensor_copy(gbo_c[:, 48:64], ghbn[:])
                nc.sync.dma_start(dbg_outs["d_gb1o"][:], gbo_c[:])
            return garz, ghn, ghbn, gxn, gb

        garz, ghn, ghbn, gxn, gb = open_groups()

        ha_rm = wk.tile([P, HA], f32, tag="ha_rm")
        nc.vector.memset(ha_rm[:], 0.0)
        hb_rm = wk.tile([P, HB], f32, tag="hb_rm")
        nc.vector.memset(hb_rm[:], 0.0)

        # ---- time loop ----
        for t in range(T_steps):
            # gumbel prefetch
            a2 = io.tile([P, K], f32, tag="a2")
            nc.sync.dma_start(a2[:], ins["gw"][t, :, :])
            a3 = io.tile([P, K], f32, tag="a3")
            nc.sync.dma_start(a3[:], ins["gwi"][t, :, :])

            # dyn features -> transposed [3, P]
            tpd = ps_t.tile([P, 512], f32, tag="tpw")
            nc.tensor.transpose(tpd[:3, 384:512], dynstage[:], ident[:])
            dynT = wk.tile([3, P], f32r, tag="dynT")
            nc.vector.tensor_copy(dynT[:], tpd[:3, 384:512])
            if dbg and t == 1:
                dynTf = wk.tile([3, P], f32, tag="dynTf")
                nc.vector.tensor_copy(dynTf[:], dynT[:])
                nc.sync.dma_start(dbg_outs["d_dynT1"][:], dynTf[:])

            # dyn matmuls close the rz / gxn groups (single-pass f32r)
            nc.tensor.matmul(garz[:, 0:512], dynT[:], wdynr[:, 0:512],
                             start=False, stop=True)
            nc.tensor.matmul(garz[:, 512:768], dynT[:], wdynr[:, 512:768],
                             start=False, stop=True)
            nc.tensor.matmul(gxn[:], dynT[:], wdynr[:, 768:G3],
                             start=False, stop=True)

            # GRU-A gates, tanh-only (sigmoid via th=tanh(x/2):
            # r*ghn == (th_r+1)*(0.5*ghn), 0.5 folded into Wha_n host-side;
            # z-blend: ha2 = an + z*(ha-an) == 0.5*((th_z+1)*(ha-an)) + an)
            thr = wk.tile([P, HA], f32, tag="thr")
            nc.scalar.activation(thr[:], garz[:, 0:HA], AF.Tanh, scale=0.5)
            thz = wk.tile([P, HA], f32, tag="thz")
            nc.scalar.activation(thz[:], garz[:, HA:768], AF.Tanh, scale=0.5)
            t1 = wk.tile([P, HA], f32, tag="t1")
            nc.vector.scalar_tensor_tensor(t1[:], thr[:], 1.0, ghn[:],
                                           op0=ALU.add, op1=ALU.mult)
            t3 = wk.tile([P, HA], f32, tag="t3")
            nc.vector.tensor_tensor(t3[:], t1[:], gxn[:], op=ALU.add)
            an = wk.tile([P, HA], f32, tag="an")
            nc.scalar.activation(an[:], t3[:], AF.Tanh)
            d = wk.tile([P, HA], f32, tag="d")
            nc.vector.tensor_tensor(d[:], ha_rm[:], an[:], op=ALU.subtract)
            zd = wk.tile([P, HA], f32, tag="zd")
            nc.vector.scalar_tensor_tensor(zd[:], thz[:], 1.0, d[:],
                                           op0=ALU.add, op1=ALU.mult)
            ha_rm = wk.tile([P, HA], f32, tag="ha_rm")
            nc.vector.scalar_tensor_tensor(ha_rm[:], zd[:], 0.5, an[:],
                                           op0=ALU.mult, op1=ALU.add)
            if dbg and t == 0:
                nc.sync.dma_start(dbg_outs["d_ha0"][:], ha_rm[:])
            if dbg and t == 1:
                garz_c = wk.tile([P, 768], f32, tag="dbg_garz")
                nc.vector.tensor_copy(garz_c[:], garz[:])
                nc.sync.dma_start(dbg_outs["d_garz1"][:], garz_c[:])
                ghn_c = wk.tile([P, HA], f32, tag="dbg_ghn")
                nc.vector.tensor_copy(ghn_c[:], ghn[:])
                nc.sync.dma_start(dbg_outs["d_ghn1"][:], ghn_c[:])
                gxn_c = wk.tile([P, HA], f32, tag="dbg_gxn")
                nc.vector.tensor_copy(gxn_c[:], gxn[:])
                nc.sync.dma_start(dbg_outs["d_gxn1"][:], gxn_c[:])
                nc.sync.dma_start(dbg_outs["d_r1"][:], r[:])
                nc.sync.dma_start(dbg_outs["d_an1"][:], an[:])
                nc.sync.dma_start(dbg_outs["d_ha1"][:], ha_rm[:])

            # ha2 -> haT: transposes back-to-back on PE, copies on
            # scalar/vector, then the gxb matmuls
            tpc = ps_t.tile([P, 512], f32, tag="tpw")
            for c in range(3):
                nc.tensor.transpose(tpc[:, c * P:(c + 1) * P],
                                    ha_rm[:, c * P:(c + 1) * P], ident[:])
            nc.scalar.copy(haTw[:, 0:P], tpc[:, 0:P])
            nc.vector.tensor_copy(haTw[:, P:2 * P], tpc[:, P:2 * P])
            nc.scalar.copy(haTw[:, 2 * P:3 * P], tpc[:, 2 * P:3 * P])
            for c in range(3):
                nc.tensor.matmul(gb[:, 0:48], haT[c], wxb[:, c * 48:(c + 1) * 48],
                                 start=False, stop=(c == 2))

            # GRU-B gates (same tanh-only scheme; 0.5 folded into whbbx_n)
            thb = wk.tile([P, 2 * HB], f32, tag="thb")
            nc.scalar.activation(thb[:], gb[:, 0:32], AF.Tanh, scale=0.5)
            t1b = wk.tile([P, HB], f32, tag="t1b")
            nc.vector.scalar_tensor_tensor(t1b[:], thb[:, 0:HB], 1.0, ghbn[:],
                                           op0=ALU.add, op1=ALU.mult)
            t2b = wk.tile([P, HB], f32, tag="t2b")
            nc.vector.tensor_tensor(t2b[:], t1b[:], gb[:, 32:48], op=ALU.add)
            nb = wk.tile([P, HB], f32, tag="nb")
            nc.scalar.activation(nb[:], t2b[:], AF.Tanh)
            db = wk.tile([P, HB], f32, tag="db")
            nc.vector.tensor_tensor(db[:], hb_rm[:], nb[:], op=ALU.subtract)
            zdb = wk.tile([P, HB], f32, tag="zdb")
            nc.vector.scalar_tensor_tensor(zdb[:], thb[:, HB:2 * HB], 1.0, db[:],
                                           op0=ALU.add, op1=ALU.mult)
            hb_rm = wk.tile([P, HB], f32, tag="hb_rm")
            nc.vector.scalar_tensor_tensor(hb_rm[:], zdb[:], 0.5, nb[:],
                                           op0=ALU.mult, op1=ALU.add)
            if dbg and t == 0:
                nc.sync.dma_start(dbg_outs["d_hb0"][:], hb_rm[:])
            if dbg and t == 1:
                gb_c = wk.tile([P, 64], f32, tag="dbg_gb")
                nc.vector.tensor_copy(gb_c[:, 0:48], gb[:])
                nc.vector.tensor_copy(gb_c[:, 48:64], ghbn[:])
                nc.sync.dma_start(dbg_outs["d_gb1"][:], gb_c[:])
                nc.sync.dma_start(dbg_outs["d_hb1"][:], hb_rm[:])

            # hb -> hbT, logits
            tpb = ps_t.tile([P, 512], f32, tag="tpw")
            nc.tensor.transpose(tpb[:HB, 384:512], hb_rm[:], ident[:])
            nc.scalar.copy(hbTa[:HB, :], tpb[:HB, 384:512])

            lps = ps_l.tile([P, 2 * K], f32, tag="lps")
            nc.tensor.matmul(lps[:], hbTa[:], w12b[:], start=True, stop=True)
            l12 = wk.tile([P, 2 * K], f32, tag="l12")
            nc.scalar.activation(l12[:], lps[:], AF.Tanh)
            lg12 = wk.tile([P, 2 * K], f32, tag="lg12")
            nc.vector.tensor_tensor(lg12[:], l12[:], g12[:], op=ALU.mult)
            logits = io.tile([P, K], f32, tag="logits")
            nc.vector.tensor_tensor(logits[:], lg12[:, 0:K], lg12[:, K:2 * K],
                                    op=ALU.add)
            nc.sync.dma_start(out_l[t, :, :], logits[:])

            # gumbel softmax expected index:
            # probs ~ exp(logits - ln(-ln u)) = exp(logits) * w,
            # w = -1/ln(u) and w*idx precomputed on host (a2, a3)
            E = wk.tile([P, K], f32, tag="E")
            nc.scalar.activation(E[:], logits[:], AF.Exp)
            Ew = wk.tile([P, K], f32, tag="Ew")
            den = wk.tile([P, 1], f32, tag="den")
            nc.vector.scalar_tensor_tensor(Ew[:], E[:], 0.0, a2[:],
                                           op0=ALU.bypass, op1=ALU.mult,
                                           accum_out=den[:])
            Escr = wk.tile([P, K], f32, tag="Escr")
            num = wk.tile([P, 1], f32, tag="num")
            nc.vector.scalar_tensor_tensor(Escr[:], E[:], 0.0, a3[:],
                                           op0=ALU.bypass, op1=ALU.mult,
                                           accum_out=num[:])
            rden = wk.tile([P, 1], f32, tag="rden")
            nc.vector.reciprocal(rden[:], den[:])
            # e = round-half-even(num/den) == jnp.round, via i32-output mult
            eint = wk.tile([P, 1], i32, tag="eint")
            nc.vector.tensor_tensor(eint[:], num[:], rden[:], op=ALU.mult)
            nc.vector.tensor_copy(dynstage[:, 2:3], eint[:])
            # s = p + e -> ring slot; col1 = p (before tau overwrites col0)
            nc.vector.tensor_tensor(ring[:, t % 16:t % 16 + 1], dynstage[:, 0:1],
                                    dynstage[:, 2:3], op=ALU.add)
            nc.vector.tensor_copy(dynstage[:, 1:2], dynstage[:, 0:1])
            # p(t+1) path
            rot = ((t + 1) % 16) * 16
            sc16 = wk.tile([P, 16], f32, tag="sc16")
            pdot = wk.tile([P, 1], f32, tag="pdot")
            nc.vector.scalar_tensor_tensor(sc16[:], lpcrot[:, rot:rot + 16], 0.0,
                                           ring[:], op0=ALU.bypass, op1=ALU.mult,
                                           accum_out=pdot[:])
            pscr = wk.tile([P, K], f32, tag="pscr")
            nc.vector.tensor_scalar(pscr[:], taub[:], pdot[:], 0.0,
                                    op0=ALU.is_le, op1=ALU.add,
                                    accum_out=dynstage[:, 0:1])

            if dbg and t == 0:
                nc.sync.dma_start(dbg_outs["d_e0"][:], dynstage[:, 2:3])
                nc.sync.dma_start(dbg_outs["d_s0"][:], ring[:, 0:1])
                nc.sync.dma_start(dbg_outs["d_p0"][:], dynstage[:, 1:2])
                nc.sync.dma_start(dbg_outs["d_p1"][:], dynstage[:, 0:1])
                nc.sync.dma_start(dbg_outs["d_pdot1"][:], pdot[:])

            # open accumulation groups for t+1 (hidden under this step's tail)
            if t + 1 < T_steps:
                garz, ghn, ghbn, gxn, gb = open_groups(dump=(dbg and t == 0))

    return nc, ins, out_l


def _pack_inputs(frames_features, lpc_coeffs, gumbel_u, Wf1, bf1, Wf2, bf2,
                 Wxa, Wha, ba, Wxb, Whb, bb, W1, b1, g1, W2, b2, g2,
                 T_steps=T):
    """Host-side packing -> list of per-core input dicts."""
    import ml_dtypes
    fp = np.float32
    bf = ml_dtypes.bfloat16
    feat = np.ascontiguousarray(frames_features, fp).reshape(R, NF)
    lpc = np.ascontiguousarray(lpc_coeffs, fp).reshape(R, M)
    u = np.ascontiguousarray(gumbel_u, fp)
    gw = (-1.0 / np.log(u[:T_steps])).astype(fp)
    gwi = (gw * np.arange(K, dtype=fp)[None, None, :]).astype(fp)

    # lpcrot[:, 16*r + j] = lpc[:, (j - r) % 16]
    lpcrot = np.empty((R, 256), fp)
    for r_ in range(16):
        for j in range(16):
            lpcrot[:, 16 * r_ + j] = lpc[:, (j - r_) % 16]

    wha_s = np.asarray(Wha, fp).copy()
    wha_s[:, 2 * 384:] *= fp(0.5)        # 0.5*gh_n for the tanh-only r-gate
    wha_p = np.concatenate([np.ascontiguousarray(wha_s[c * P:(c + 1) * P, :])
                            for c in range(3)], axis=1)          # [128, 3*1152]
    # wxb chunk c: [rz 32 | n 16]
    wxb_p = np.concatenate(
        [np.concatenate([np.asarray(Wxb, fp)[c * P:(c + 1) * P, 0:32],
                         np.asarray(Wxb, fp)[c * P:(c + 1) * P, 32:48]], axis=1)
         for c in range(3)], axis=1)                             # [128, 144]
    statics = {
        "wf1b": np.concatenate([np.asarray(Wf1, fp), np.asarray(bf1, fp)[None, :]], 0),
        "wf2": np.asarray(Wf2, fp),
        "bf2": np.asarray(bf2, fp)[None, :],
        "wxac": np.ascontiguousarray(np.asarray(Wxa, fp)[:COND, :]),
        "ba": np.asarray(ba, fp)[None, :],
        "wdynr": _wdynr(np.asarray(Wxa, fp)),
        "wha": wha_p,
        "wxb": wxb_p,
        "whbbx": np.concatenate([
            np.concatenate([np.asarray(Whb, fp)[:, 0:32],
                            np.asarray(bb, fp)[None, 0:32]], 0),
            np.concatenate([np.zeros((HB, HB), fp),
                            np.asarray(bb, fp)[None, 32:48]], 0),
            np.concatenate([np.asarray(Whb, fp)[:, 32:48] * fp(0.5),
                            np.zeros((1, HB), fp)], 0)], axis=1),
        "w12b": np.concatenate([
            np.concatenate([np.asarray(W1, fp), np.asarray(W2, fp)], axis=1),
            np.concatenate([np.asarray(b1, fp), np.asarray(b2, fp)])[None, :]],
            0),
        "g12": np.repeat(np.concatenate([np.asarray(g1, fp), np.asarray(g2, fp)])[None, :], P, 0),
        "idxb": np.repeat(np.arange(K, dtype=fp)[None, :], P, 0),
        "ident": np.eye(P, dtype=fp),
        "identr": np.eye(P, dtype=fp),
        "onesrow": np.ones((1, P), fp),
        "onesrowr": np.ones((1, P), fp),
        "zhaT": np.zeros((P, 3 * P), fp),
        "zhbT": np.zeros((HB, P), fp),
        "taub": np.repeat(_tau_table()[None, :], P, 0),
    }
    per_core = []
    for c in range(N_CORES):
        rs = slice(c * P, (c + 1) * P)
        m = dict(statics)
        m["feat"] = np.ascontiguousarray(feat[rs])
        m["lpcrot"] = np.ascontiguousarray(lpcrot[rs])
        m["gw"] = np.ascontiguousarray(gw[:, rs, :])
        m["gwi"] = np.ascontiguousarray(gwi[:, rs, :])
        per_core.append(m)
    return per_core


def _wdynr(Wxa):
    """[3, 3H] fp32: [w_p, w_s, w_s+w_e] rows for the f32r dyn matmul."""
    fp = np.float32
    wd = Wxa[COND:COND + 3, :].astype(fp).copy()
    wd[2] = (wd[1] + wd[2]).astype(fp)
    return wd


def _wdyn2(Wxa):
    """[6, 3H]: bf16 hi/lo split of [w_p, w_s, w_s+w_e] (exact-bf16 dyn matmul)."""
    import ml_dtypes
    fp = np.float32
    wd = Wxa[COND:COND + 3, :].astype(fp).copy()
    wd[2] = (wd[1] + wd[2]).astype(fp)
    hi = wd.astype(ml_dtypes.bfloat16).astype(fp)
    lo = (wd - hi).astype(ml_dtypes.bfloat16).astype(fp)
    return np.concatenate([hi, lo], 0)


def _tau_table():
    """tau[k] = smallest float32 x with mu_law_p(x) >= k+1 (k=0..254);
    tau[255] = +inf sentinel. p(x) = sum_k [x >= tau_k]."""
    fp = np.float32

    def p_of(x):
        x = np.asarray(x, fp)
        xc = np.clip(x, fp(-1.0), fp(1.0)).astype(fp)
        ln_mu1 = np.log(fp(256.0)).astype(fp)
        y = (np.sign(xc) * np.log1p(fp(255.0) * np.abs(xc)) / ln_mu1).astype(fp)
        v = ((y + fp(1.0)) * fp(0.5) * fp(256.0)).astype(fp)
        return np.clip(np.floor(v), 0.0, 255.0)

    def f2i(x):
        b = np.asarray(x, np.float32).view(np.int32)
        return np.where(b < 0, np.int32(-2147483648) - b, b).astype(np.int64)

    def i2f(i):
        i = np.asarray(i, np.int64)
        b = np.where(i < 0, -2147483648 - i, i).astype(np.int32)
        return b.view(np.float32)

    ks = np.arange(1, 256)
    lo = np.full(255, f2i(np.float32(-1.5)), np.int64)
    hi = np.full(255, f2i(np.float32(1.5)), np.int64)
    for _ in range(40):
        mid = (lo + hi) // 2
        ge = p_of(i2f(mid)) >= ks
        hi = np.where(ge, mid, hi)
        lo = np.where(ge, lo, mid)
    tau = i2f(hi).astype(fp)
    out = np.empty(256, fp)
    out[:255] = tau
    out[255] = np.float32(3.0e38)
    return out


_CACHE = {}


def _ensure_devices():
    import jax
    try:
        if len(jax.devices()) >= N_CORES:
            return
    except Exception:
        pass
    jax.config.update("jax_platforms", "axon,cpu")
    import jax.extend.backend as _jeb
    _jeb.clear_backends()
    assert len(jax.devices()) >= N_CORES, (
        f"need {N_CORES} NeuronCores, visible: {jax.devices()}")


def _get_nc(T_steps):
    if T_steps not in _CACHE:
        nc, ins, out_l = _build(T_steps)
        nc.compile()
        _CACHE[T_steps] = nc
    return _CACHE[T_steps]


def kernel(**inputs):
    _ensure_devices()
    nc = _get_nc(T)
    per_core = _pack_inputs(**inputs)
    res = run_bass_kernel_spmd(nc, per_core, list(range(N_CORES)))
    shards = [res.results[c]["logits"] for c in range(N_CORES)]   # each [T,128,K]
    logits_seq = np.concatenate(shards, axis=1)                   # [T, R, K]
    out = logits_seq.transpose(1, 0, 2).reshape(B, F * T, K)
    return np.ascontiguousarray(out, dtype=np.float32)
